# revision 1
# baseline (speedup 1.0000x reference)
"""GPT-2 (L=8, D=1024, H=16, V=50257, B=4, T=1024) forward on 8 TRN2 NeuronCores.

Sharding: core c handles batch b=c//2, sequence half h=c%2 (512 tokens).
Weights replicated (bf16). Per layer, K/V for the half-sequence are exchanged
between the two cores of a batch-pair with an AllGather, so every core attends
over the full 1024-token causal context for its own 512 queries.

Activation layout on-chip: x is kept transposed, [d (8x128 partitions), tok],
so every projection matmul uses weights as the stationary operand and never
needs an activation transpose. V is produced in [tok, d] layout directly, and
augmented with a ones-column per head so the AV matmul also produces the
softmax denominators (V_aug is [tok, 16*65]).
"""

import os
import sys
import types
import contextlib

import numpy as np
import ml_dtypes

import concourse.bass as bass
import concourse.mybir as mybir
import concourse.tile as tile
from concourse import bacc
from concourse.bass_utils import run_bass_kernel_spmd

f32 = mybir.dt.float32
bf16 = mybir.dt.bfloat16
AF = mybir.ActivationFunctionType
OP = mybir.AluOpType

L, D, H, V, DFF = 8, 1024, 16, 50257, 4096
HS = D // H          # 64
B, T = 4, 1024
TPC = 512            # tokens per core
P = 128
DC = D // P          # 8 d-chunks
FC = DFF // P        # 32 dff-chunks
NVC = (V + 511) // 512   # 99 vocab chunks
EPS = 1e-5

K_SZ = DC * P * TPC            # K staging elems per core
V_SZ = 4 * P * (H * (HS + 1))  # V_aug staging elems per core (4 tok chunks x 128 x 1040)
KV_SZ = K_SZ + V_SZ
VW = H * (HS + 1)              # 1040

LAST_EXEC_NS = None
_CACHE = {}


def _install_ntff_hook():
    """Provide antenv.axon_hooks if the image lacks it, so trace=True works."""
    try:
        import antenv
        try:
            from antenv import axon_hooks  # noqa: F401
            return
        except ImportError:
            pass
        hooks_mod = types.ModuleType("antenv.axon_hooks")
        _hook = [None]
        hooks_mod.set_axon_ntff_profile_hook = lambda h: _hook.__setitem__(0, h)
        hooks_mod.get_axon_ntff_profile_hook = lambda: _hook[0]
        sys.modules["antenv.axon_hooks"] = hooks_mod
        antenv.axon_hooks = hooks_mod
        from trn_agent_boot.trn_boot import _ntff_profile_via_ctypes
        hooks_mod.set_axon_ntff_profile_hook(
            _ntff_profile_via_ctypes("/opt/axon/libaxon_pjrt.so"))
    except Exception:
        pass


def _layernorm(nc, pool, pstat, pmm, small, ones128b, ones1, eps_t, x, w_pc, b_pc, out_bf, nm):
    """LN over d (partitions x chunks) of x [128, DC, 512] fp32 -> out_bf bf16."""
    xbf = pool.tile([P, DC, TPC], bf16, tag="xbf", name=f"xbf_{nm}")
    sqbf = pool.tile([P, DC, TPC], bf16, tag="sqbf", name=f"sqbf_{nm}")
    nc.vector.tensor_copy(xbf[:], x[:])
    nc.vector.tensor_mul(sqbf[:], xbf[:], xbf[:])
    sx = pstat.tile([1, TPC], f32, tag="stat", name=f"sx_{nm}")
    sq = pstat.tile([1, TPC], f32, tag="stat", name=f"sq_{nm}")
    for c in range(DC):
        nc.tensor.matmul(sx[:], ones128b[:], xbf[:, c, :], start=(c == 0), stop=(c == DC - 1))
    for c in range(DC):
        nc.tensor.matmul(sq[:], ones128b[:], sqbf[:, c, :], start=(c == 0), stop=(c == DC - 1))
    mu = small.tile([1, TPC], f32, tag="sm", name=f"mu_{nm}")
    ex2 = small.tile([1, TPC], f32, tag="sm", name=f"ex2_{nm}")
    nc.vector.tensor_scalar_mul(mu[:], sx[:], 1.0 / D)
    nc.vector.tensor_scalar_mul(ex2[:], sq[:], 1.0 / D)
    var = small.tile([1, TPC], f32, tag="sm", name=f"var_{nm}")
    nc.vector.tensor_mul(var[:], mu[:], mu[:])
    nc.vector.tensor_sub(var[:], ex2[:], var[:])
    nc.scalar.activation(var[:], var[:], AF.Sqrt, bias=eps_t[:], scale=1.0)
    rstd = small.tile([1, TPC], f32, tag="sm", name=f"rstd_{nm}")
    nc.vector.reciprocal(rstd[:], var[:])
    murstd = small.tile([1, TPC], f32, tag="sm", name=f"murstd_{nm}")
    nc.vector.tensor_mul(murstd[:], mu[:], rstd[:])
    rsb = pmm.tile([P, TPC], f32, tag="mm", name=f"rsb_{nm}")
    msb = pmm.tile([P, TPC], f32, tag="mm", name=f"msb_{nm}")
    nc.tensor.matmul(rsb[:], ones1[:], rstd[:], start=True, stop=True)
    nc.tensor.matmul(msb[:], ones1[:], murstd[:], start=True, stop=True)
    nc.vector.tensor_mul(out_bf[:], x[:], rsb[:, None, :].to_broadcast([P, DC, TPC]))
    nc.vector.tensor_sub(out_bf[:], out_bf[:], msb[:, None, :].to_broadcast([P, DC, TPC]))
    for c in range(DC):
        nc.vector.scalar_tensor_tensor(
            out_bf[:, c, :], out_bf[:, c, :], w_pc[:, c], b_pc[:, c].to_broadcast([P, TPC]),
            op0=OP.mult, op1=OP.add)


def _build():
    nc = bacc.Bacc(None, target_bir_lowering=False, debug=False)

    xembT = nc.dram_tensor("xembT", [D, TPC], f32, kind="ExternalInput")
    wq = nc.dram_tensor("wq", [L, P, DC, D], bf16, kind="ExternalInput")
    wk = nc.dram_tensor("wk", [L, P, DC, D], bf16, kind="ExternalInput")
    wv = nc.dram_tensor("wv", [L, P, DC, D], bf16, kind="ExternalInput")
    wo = nc.dram_tensor("wo", [L, P, DC, D], bf16, kind="ExternalInput")
    w1 = nc.dram_tensor("w1", [L, FC, P, DC, P], bf16, kind="ExternalInput")
    w2 = nc.dram_tensor("w2", [L, 4, DC, P, 8, P], bf16, kind="ExternalInput")
    wlm = nc.dram_tensor("wlm", [NVC, P, DC, 512], bf16, kind="ExternalInput")
    ln1w = nc.dram_tensor("ln1w", [L, P, DC], f32, kind="ExternalInput")
    ln1b = nc.dram_tensor("ln1b", [L, P, DC], f32, kind="ExternalInput")
    ln2w = nc.dram_tensor("ln2w", [L, P, DC], f32, kind="ExternalInput")
    ln2b = nc.dram_tensor("ln2b", [L, P, DC], f32, kind="ExternalInput")
    lnfw = nc.dram_tensor("lnfw", [P, DC], f32, kind="ExternalInput")
    lnfb = nc.dram_tensor("lnfb", [P, DC], f32, kind="ExternalInput")
    bo_d = nc.dram_tensor("bo", [L, P, DC], f32, kind="ExternalInput")
    b1_d = nc.dram_tensor("b1", [L, P, FC], f32, kind="ExternalInput")
    b2_d = nc.dram_tensor("b2", [L, P, DC], f32, kind="ExternalInput")
    blm_d = nc.dram_tensor("blm", [V], f32, kind="ExternalInput")
    mask_d = nc.dram_tensor("mask", [P, 2 * DC // 2, TPC], bf16, kind="ExternalInput")
    out_d = nc.dram_tensor("out", [TPC, V], f32, kind="ExternalOutput")

    kv_loc = nc.dram_tensor("kv_loc", [KV_SZ], bf16)
    kv_gat = nc.dram_tensor("kv_gat", [2, KV_SZ], bf16)
    groups = [[0, 1], [2, 3], [4, 5], [6, 7]]

    with tile.TileContext(nc) as tc:
        with (
            tc.tile_pool(name="pool", bufs=1) as pool,
            tc.tile_pool(name="wpool", bufs=2) as wpool,
            tc.tile_pool(name="abf", bufs=4) as abf,
            tc.tile_pool(name="sexp_p", bufs=2) as sexp_p,
            tc.tile_pool(name="small", bufs=5) as small,
            tc.tile_pool(name="lnp", bufs=4) as lnp,
            tc.tile_pool(name="outp", bufs=3) as outp,
            tc.tile_pool(name="pmm", bufs=6, space="PSUM") as pmm,
            tc.tile_pool(name="pstat", bufs=2, space="PSUM") as pstat,
        ):
            # ---- persistent tiles
            x = pool.tile([P, DC, TPC], f32, name="x")
            kfull = pool.tile([P, 2, DC, TPC], bf16, name="kfull")
            vfull = pool.tile([P, 2, 4, VW], bf16, name="vfull")
            mask = pool.tile([P, DC, TPC], bf16, name="mask")
            ones128b = pool.tile([P, 1], bf16, name="ones128b")
            ones1 = pool.tile([1, P], f32, name="ones1")
            nc.vector.memset(ones128b[:], 1.0)
            nc.vector.memset(ones1[:], 1.0)
            eps_t = pool.tile([1, 1], f32, name="eps_t")
            nc.vector.memset(eps_t[:], EPS)
            nc.sync.dma_start(mask[:], mask_d[:])
            nc.sync.dma_start(x[:], xembT.rearrange("(c p) t -> p c t", p=P))
            r = pool.tile([P, 8, TPC], bf16, name="r")

            def psum_mm(name):
                return pmm.tile([P, TPC], f32, tag="mm", name=name)

            def ln(xin, w_pc, b_pc, out_bf, nm):
                _layernorm(nc, pool, pstat, pmm, small, ones128b, ones1, eps_t,
                           xin, w_pc, b_pc, out_bf, nm)

            def ln_params(wd, bd, li, nm):
                wt = lnp.tile([P, DC, 1], f32, tag="lnw", name=f"lnw_{nm}")
                bt = lnp.tile([P, DC, 1], f32, tag="lnb", name=f"lnb_{nm}")
                src_w = wd[li] if li is not None else wd
                src_b = bd[li] if li is not None else bd
                nc.sync.dma_start(wt[:], src_w[:, :, None])
                nc.sync.dma_start(bt[:], src_b[:, :, None])
                return wt, bt

            for li in range(L):
                # ---------- LN1 ----------
                w_pc, b_pc = ln_params(ln1w, ln1b, li, f"1_{li}")
                hbf = abf.tile([P, DC, TPC], bf16, tag="a", name=f"hbf_{li}")
                ln(x, w_pc, b_pc, hbf, f"l1_{li}")

                # ---------- K, V projections first (feed the collective) ----
                wk_t = wpool.tile([P, DC, D], bf16, tag="w", name=f"wk_{li}")
                nc.sync.dma_start(wk_t[:], wk[li])
                kst = abf.tile([P, DC, TPC], bf16, tag="a", name=f"kst_{li}")
                for m in range(DC):
                    ps = psum_mm(f"kps_{li}_{m}")
                    for c in range(DC):
                        nc.tensor.matmul(ps[:], wk_t[:, c, m * P:(m + 1) * P],
                                         hbf[:, c, :], start=(c == 0), stop=(c == DC - 1))
                    nc.scalar.activation(kst[:, m, :], ps[:], AF.Copy)

                wv_t = wpool.tile([P, DC, D], bf16, tag="w", name=f"wv_{li}")
                nc.sync.dma_start(wv_t[:], wv[li])
                vst = abf.tile([P, 4, VW], bf16, tag="a", name=f"vst_{li}")
                nc.vector.memset(vst[:], 1.0)
                for tc4 in range(4):
                    for mh in range(2):
                        ps = psum_mm(f"vps_{li}_{tc4}_{mh}")
                        for c in range(DC):
                            nc.tensor.matmul(
                                ps[:], hbf[:, c, tc4 * P:(tc4 + 1) * P],
                                wv_t[:, c, mh * 512:(mh + 1) * 512],
                                start=(c == 0), stop=(c == DC - 1))
                        dst = vst[:, tc4, :].rearrange("p (h e) -> p h e", e=HS + 1)
                        nc.vector.tensor_copy(
                            dst[:, mh * 8:(mh + 1) * 8, 0:HS],
                            ps[:].rearrange("p (h e) -> p h e", e=HS))
                # stage K/V to DRAM and gather
                nc.sync.dma_start(
                    kv_loc[0:K_SZ].rearrange("(p c t) -> p c t", c=DC, t=TPC), kst[:])
                nc.sync.dma_start(
                    kv_loc[K_SZ:KV_SZ].rearrange("(p c t) -> p c t", c=4, t=VW), vst[:])
                nc.gpsimd.collective_compute(
                    "AllGather", OP.bypass, replica_groups=groups,
                    ins=[kv_loc[:]], outs=[kv_gat[:]])

                # ---------- Q projection (overlaps the collective) --------
                wq_t = wpool.tile([P, DC, D], bf16, tag="w", name=f"wq_{li}")
                nc.sync.dma_start(wq_t[:], wq[li])
                qbf = abf.tile([P, DC, TPC], bf16, tag="a", name=f"qbf_{li}")
                for m in range(DC):
                    ps = psum_mm(f"qps_{li}_{m}")
                    for c in range(DC):
                        nc.tensor.matmul(ps[:], wq_t[:, c, m * P:(m + 1) * P],
                                         hbf[:, c, :], start=(c == 0), stop=(c == DC - 1))
                    nc.scalar.activation(qbf[:, m, :], ps[:], AF.Copy)

                # ---------- gathered KV back to SBUF ----------------------
                for sg in range(2):
                    nc.sync.dma_start(
                        kfull[:, sg], kv_gat[sg, 0:K_SZ].rearrange("(p c t) -> p c t", c=DC, t=TPC))
                    nc.sync.dma_start(
                        vfull[:, sg], kv_gat[sg, K_SZ:KV_SZ].rearrange("(p c t) -> p c t", c=4, t=VW))

                # ---------- attention ---------------------------------------
                obf = abf.tile([P, DC, TPC], bf16, tag="a", name=f"obf_{li}")
                for h in range(H):
                    hp = (h % 2) * HS
                    hc = h // 2
                    sexp = sexp_p.tile([P, DC, TPC], bf16, tag="sexp", name=f"sexp_{li}_{h}")
                    for kt in range(DC):
                        sl, tl = kt // 4, (kt % 4) * P
                        ps = psum_mm(f"sps_{li}_{h}_{kt}")
                        nc.tensor.matmul(
                            ps[:], kfull[hp:hp + HS, sl, hc, tl:tl + P],
                            qbf[hp:hp + HS, hc, :], start=True, stop=True)
                        nc.scalar.activation(sexp[:, kt, :], ps[:], AF.Exp, scale=HS ** -0.5)
                    nc.vector.tensor_mul(sexp[:], sexp[:], mask[:])
                    ops = psum_mm(f"ops_{li}_{h}")
                    for kt in range(DC):
                        nc.tensor.matmul(
                            ops[0:HS + 1, :], vfull[:, kt // 4, kt % 4, h * 65:h * 65 + 65],
                            sexp[:, kt, :], start=(kt == 0), stop=(kt == DC - 1))
                    rc = small.tile([1, TPC], f32, tag="rcb", name=f"rc_{li}_{h}")
                    nc.vector.reciprocal(rc[:], ops[HS:HS + 1, :])
                    bc = psum_mm(f"bcp_{li}_{h}")
                    nc.tensor.matmul(bc[0:HS, :], ones1[:, 0:HS], rc[:], start=True, stop=True)
                    bcs = small.tile([HS, TPC], f32, tag="rcb", name=f"bcs_{li}_{h}")
                    nc.vector.tensor_copy(bcs[:], bc[0:HS, :])
                    nc.vector.tensor_mul(obf[hp:hp + HS, hc, :], ops[0:HS, :], bcs[:])

                # ---------- output projection + residual --------------------
                wo_t = wpool.tile([P, DC, D], bf16, tag="w", name=f"wo_{li}")
                nc.sync.dma_start(wo_t[:], wo[li])
                bo_t = lnp.tile([P, DC, 1], f32, tag="bias", name=f"bo_{li}")
                nc.sync.dma_start(bo_t[:], bo_d[li][:, :, None])
                for m in range(DC):
                    ps = psum_mm(f"ops2_{li}_{m}")
                    for c in range(DC):
                        nc.tensor.matmul(ps[:], wo_t[:, c, m * P:(m + 1) * P],
                                         obf[:, c, :], start=(c == 0), stop=(c == DC - 1))
                    nc.vector.scalar_tensor_tensor(
                        x[:, m, :], ps[:], bo_t[:, m], x[:, m, :], op0=OP.add, op1=OP.add)

                # ---------- LN2 + MLP ----------------------------------------
                w_pc2, b_pc2 = ln_params(ln2w, ln2b, li, f"2_{li}")
                h2 = abf.tile([P, DC, TPC], bf16, tag="a", name=f"h2_{li}")
                ln(x, w_pc2, b_pc2, h2, f"l2_{li}")

                b1_t = lnp.tile([P, FC, 1], f32, tag="b1", name=f"b1_{li}")
                nc.sync.dma_start(b1_t[:], b1_d[li][:, :, None])
                b2_t = lnp.tile([P, DC, 1], f32, tag="bias", name=f"b2_{li}")
                nc.sync.dma_start(b2_t[:], b2_d[li][:, :, None])
                for qr in range(4):
                    for mfl in range(8):
                        mf = qr * 8 + mfl
                        w1_t = wpool.tile([P, DC, P], bf16, tag="w1", name=f"w1_{li}_{mf}")
                        nc.sync.dma_start(w1_t[:], w1[li, mf])
                        ps = psum_mm(f"mps_{li}_{mf}")
                        for c in range(DC):
                            nc.tensor.matmul(ps[:], w1_t[:, c, :], h2[:, c, :],
                                             start=(c == 0), stop=(c == DC - 1))
                        nc.scalar.activation(r[:, mfl, :], ps[:], AF.Relu, bias=b1_t[:, mf], scale=1.0)
                    for m in range(DC):
                        w2_t = wpool.tile([P, 8, P], bf16, tag="w2", name=f"w2_{li}_{qr}_{m}")
                        nc.sync.dma_start(w2_t[:], w2[li, qr, m])
                        ps = psum_mm(f"m2ps_{li}_{qr}_{m}")
                        for c in range(8):
                            nc.tensor.matmul(ps[:], w2_t[:, c, :], r[:, c, :],
                                             start=(c == 0), stop=(c == 7))
                        if qr == 0:
                            nc.vector.scalar_tensor_tensor(
                                x[:, m, :], ps[:], b2_t[:, m], x[:, m, :], op0=OP.add, op1=OP.add)
                        else:
                            nc.vector.tensor_add(x[:, m, :], x[:, m, :], ps[:])

            # ---------- final LN + LM head ----------------------------------
            w_pcf, b_pcf = ln_params(lnfw, lnfb, None, "f")
            xf = abf.tile([P, DC, TPC], bf16, tag="a", name="xf")
            ln(x, w_pcf, b_pcf, xf, "lf")

            for vc in range(NVC):
                nv = min(512, V - vc * 512)
                wl_t = wpool.tile([P, DC, 512], bf16, tag="w", name=f"wlm_{vc}")
                nc.sync.dma_start(wl_t[:], wlm[vc])
                bl = small.tile([1, 512], f32, tag="rcb", name=f"bl_{vc}")
                nc.sync.dma_start(bl[:, 0:nv], blm_d[None, vc * 512:vc * 512 + nv])
                bcp = psum_mm(f"blmp_{vc}")
                nc.tensor.matmul(bcp[:, 0:nv], ones1[:], bl[:, 0:nv], start=True, stop=True)
                bls = outp.tile([P, 512], f32, tag="o", name=f"bls_{vc}")
                nc.vector.tensor_copy(bls[:, 0:nv], bcp[:, 0:nv])
                for tc4 in range(4):
                    ps = psum_mm(f"lmps_{vc}_{tc4}")
                    for c in range(DC):
                        nc.tensor.matmul(ps[:, 0:nv], xf[:, c, tc4 * P:(tc4 + 1) * P],
                                         wl_t[:, c, 0:nv], start=(c == 0), stop=(c == DC - 1))
                    ot = outp.tile([P, 512], f32, tag="o", name=f"ot_{vc}_{tc4}")
                    nc.vector.tensor_add(ot[:, 0:nv], ps[:, 0:nv], bls[:, 0:nv])
                    nc.sync.dma_start(
                        out_d[tc4 * P:(tc4 + 1) * P, vc * 512:vc * 512 + nv], ot[:, 0:nv])

    nc.compile()
    return nc


def kernel(**inputs):
    global LAST_EXEC_NS
    _install_ntff_hook()
    if "nc" not in _CACHE:
        _CACHE["nc"] = _build()
    nc = _CACHE["nc"]

    gi = {k: np.asarray(v) for k, v in inputs.items()}
    idx = gi["idx"].astype(np.int64)
    xemb = gi["wte"][idx] + gi["wpe"][:T][None, :, :]      # [B, T, D] fp32

    def cast(a):
        return np.ascontiguousarray(a.astype(ml_dtypes.bfloat16))

    def pack_sq(w):   # [L, 1024, N] -> [L, 128, 8, N]
        Lw, Kw, Nw = w.shape
        return np.ascontiguousarray(
            w.reshape(Lw, DC, P, Nw).transpose(0, 2, 1, 3).astype(ml_dtypes.bfloat16))

    w1p = gi["w1"].reshape(L, DC, P, FC, P).transpose(0, 3, 2, 1, 4)   # [L,FC,P,DC,P]
    w1p = np.ascontiguousarray(w1p.astype(ml_dtypes.bfloat16))
    w2p = gi["w2"].reshape(L, 4, 8, P, DC, P).transpose(0, 1, 4, 3, 2, 5)  # [L,4,DC,P,8,P]
    w2p = np.ascontiguousarray(w2p.astype(ml_dtypes.bfloat16))
    wlmp = np.zeros((D, NVC * 512), np.float32)
    wlmp[:, :V] = gi["wlm"]
    wlmp = wlmp.reshape(DC, P, NVC, 512).transpose(2, 1, 0, 3)         # [NVC,P,DC,512]
    wlmp = np.ascontiguousarray(wlmp.astype(ml_dtypes.bfloat16))

    def packv(v):  # [.., N] -> [.., P, N//P] (chunk-major per partition)
        v = np.asarray(v, np.float32)
        nch = v.shape[-1] // P
        return np.ascontiguousarray(
            v.reshape(v.shape[:-1] + (nch, P)).swapaxes(-1, -2))

    shared = dict(
        wq=pack_sq(gi["wq"]), wk=pack_sq(gi["wk"]), wv=pack_sq(gi["wv"]), wo=pack_sq(gi["wo"]),
        w1=w1p, w2=w2p, wlm=wlmp,
        ln1w=packv(gi["ln1_w"]), ln1b=packv(gi["ln1_b"]),
        ln2w=packv(gi["ln2_w"]), ln2b=packv(gi["ln2_b"]),
        lnfw=packv(gi["lnf_w"]), lnfb=packv(gi["lnf_b"]),
        bo=packv(gi["bo"]), b1=packv(gi["b1"]), b2=packv(gi["b2"]),
        blm=np.ascontiguousarray(gi["blm"], np.float32),
    )

    in_maps = []
    for c in range(8):
        b, half = c // 2, c % 2
        q0 = half * TPC
        sl = slice(q0, q0 + TPC)
        m = np.zeros((P, DC, TPC), np.float32)
        k_abs = np.arange(P)[:, None] + (np.arange(DC) * P)[None, :]   # [P, DC]
        q_abs = q0 + np.arange(TPC)
        m[:] = (k_abs[:, :, None] <= q_abs[None, None, :]).astype(np.float32)
        im = dict(shared)
        im["xembT"] = np.ascontiguousarray(xemb[b, sl].T, dtype=np.float32)
        im["mask"] = m.astype(ml_dtypes.bfloat16)
        in_maps.append(im)

    res = run_bass_kernel_spmd(nc, in_maps, list(range(8)),
                               trace=bool(os.environ.get("BASS_TRACE")))
    LAST_EXEC_NS = res.exec_time_ns

    out = np.empty((B, T, V), np.float32)
    for c in range(8):
        b, half = c // 2, c % 2
        out[b, half * TPC:(half + 1) * TPC] = res.results[c]["out"]
    return out



# revision 9
# speedup vs baseline: 1.2052x; 1.2052x over previous
"""GPT-2 (L=8, D=1024, H=16, V=50257, B=4, T=1024) forward on 8 TRN2 NeuronCores.

Sharding: core c handles batch b=c//2, sequence half h=c%2 (512 tokens).
Weights replicated (bf16). Per layer, K/V are exchanged between the two cores
of a batch-pair with an AllReduce(add); each core recovers the peer half by
subtracting its own contribution (bf16 sub). Attention chunk order is
core-relative: chunks 0-3 = local keys (direct from SBUF, no collective wait),
chunks 4-7 = peer keys. Causality is data-driven: a diagonal [128,4,512] mask
(identical on all cores) for the local half, and a per-core exp bias
(0 or -60000) that zeroes the whole peer half on first-half cores.

LN weights/biases are folded into the adjacent projection weights host-side,
so on-chip LN is a pure (x-mu)*rstd; stats are accumulated chunk-by-chunk as
the residual stream is produced. LM head runs in vocab groups of 6 sharing
the stationary activations across 6 PSUM banks, bf16 output (host upcasts).
"""

import os
import sys
import types
from contextlib import ExitStack

import numpy as np
import ml_dtypes

import concourse.bass as bass
import concourse.mybir as mybir
import concourse.tile as tile
from concourse import bacc
from concourse.bass_utils import run_bass_kernel_spmd

f32 = mybir.dt.float32
bf16 = mybir.dt.bfloat16
AF = mybir.ActivationFunctionType
OP = mybir.AluOpType

L, D, H, V, DFF = 8, 1024, 16, 50257, 4096
HS = D // H          # 64
B, T = 4, 1024
TPC = 512            # tokens per core
P = 128
DC = D // P          # 8 d-chunks
FC = DFF // P        # 32 dff-chunks
NVC = (V + 511) // 512   # 99 vocab chunks
GV = 6               # lm-head vocab chunks per group
NG = (NVC + GV - 1) // GV        # 17 groups
NVC2 = NG * GV                   # 102 (padded)
VPAD = NVC2 * 512
EPS = 1e-5
VW = H * (HS + 1)    # 1040

K_SZ = P * DC * TPC           # 524288
V_SZ = P * 4 * VW             # 532480

LAST_EXEC_NS = None
_CACHE = {}


def _install_ntff_hook():
    """Provide antenv.axon_hooks if the image lacks it, so trace=True works."""
    try:
        import antenv
        try:
            from antenv import axon_hooks  # noqa: F401
            return
        except ImportError:
            pass
        hooks_mod = types.ModuleType("antenv.axon_hooks")
        _hook = [None]
        hooks_mod.set_axon_ntff_profile_hook = lambda h: _hook.__setitem__(0, h)
        hooks_mod.get_axon_ntff_profile_hook = lambda: _hook[0]
        sys.modules["antenv.axon_hooks"] = hooks_mod
        antenv.axon_hooks = hooks_mod
        from trn_agent_boot.trn_boot import _ntff_profile_via_ctypes
        hooks_mod.set_axon_ntff_profile_hook(
            _ntff_profile_via_ctypes("/opt/axon/libaxon_pjrt.so"))
    except Exception:
        pass


def _build():
    nc = bacc.Bacc(None, target_bir_lowering=False, debug=False)

    xembT = nc.dram_tensor("xembT", [D, TPC], f32, kind="ExternalInput")
    wq = nc.dram_tensor("wq", [L, P, DC, D], bf16, kind="ExternalInput")
    wk = nc.dram_tensor("wk", [L, P, DC, D], bf16, kind="ExternalInput")
    wv = nc.dram_tensor("wv", [L, P, DC, D], bf16, kind="ExternalInput")
    wo = nc.dram_tensor("wo", [L, P, DC, D], bf16, kind="ExternalInput")
    w1 = nc.dram_tensor("w1", [L, FC, P, DC, P], bf16, kind="ExternalInput")
    w2 = nc.dram_tensor("w2", [L, DC, P, FC, P], bf16, kind="ExternalInput")
    wlm = nc.dram_tensor("wlm", [NG, P, DC, GV * 512], bf16, kind="ExternalInput")
    bq_d = nc.dram_tensor("bq", [L, P, DC], f32, kind="ExternalInput")
    bk_d = nc.dram_tensor("bk", [L, P, DC], f32, kind="ExternalInput")
    bo_d = nc.dram_tensor("bo", [L, P, DC], f32, kind="ExternalInput")
    b1_d = nc.dram_tensor("b1", [L, P, FC], f32, kind="ExternalInput")
    b2_d = nc.dram_tensor("b2", [L, P, DC], f32, kind="ExternalInput")
    blm_d = nc.dram_tensor("blm", [VPAD], f32, kind="ExternalInput")
    maskl_d = nc.dram_tensor("maskl", [P, 4, TPC], bf16, kind="ExternalInput")
    pbias_d = nc.dram_tensor("pbias", [P, 1], f32, kind="ExternalInput")
    out_d = nc.dram_tensor("out", [TPC, V], bf16, kind="ExternalOutput")

    kvloc_k = nc.dram_tensor("kvloc_k", [K_SZ], bf16)
    kvred_k = nc.dram_tensor("kvred_k", [K_SZ], bf16)
    kvloc_v = nc.dram_tensor("kvloc_v", [V_SZ], bf16)
    kvred_v = nc.dram_tensor("kvred_v", [V_SZ], bf16)
    groups = [[0, 1], [2, 3], [4, 5], [6, 7]]

    with tile.TileContext(nc) as tc:
        with (
            tc.tile_pool(name="cpool", bufs=1) as cpool,
            tc.tile_pool(name="csm", bufs=2) as csm,
        ):
            # ---- persistent / common tiles
            x = cpool.tile([P, DC, TPC], f32, name="x")
            xf = cpool.tile([P, DC, TPC], bf16, name="xf")
            maskl = cpool.tile([P, 4, TPC], bf16, name="maskl")
            pb = cpool.tile([P, 1], f32, name="pb")
            ones1 = cpool.tile([1, P], f32, name="ones1")
            ones128b = cpool.tile([P, 1], bf16, name="ones128b")
            eps_t = cpool.tile([1, 1], f32, name="eps_t")
            nc.vector.memset(ones1[:], 1.0)
            nc.vector.memset(ones128b[:], 1.0)
            nc.vector.memset(eps_t[:], EPS)
            nc.sync.dma_start(maskl[:], maskl_d[:])
            nc.sync.dma_start(pb[:], pbias_d[:])
            nc.sync.dma_start(x[:], xembT.rearrange("(c p) t -> p c t", p=P))

            lstack = ExitStack()
            lpool = lstack.enter_context(tc.tile_pool(name="lpool", bufs=1))
            wbig = lstack.enter_context(tc.tile_pool(name="wbig", bufs=2))
            w1p = lstack.enter_context(tc.tile_pool(name="w1p", bufs=3))
            w2p = lstack.enter_context(tc.tile_pool(name="w2p", bufs=2))
            sexpp = lstack.enter_context(tc.tile_pool(name="sexpp", bufs=4))
            sumc = lstack.enter_context(tc.tile_pool(name="sumc", bufs=2))
            vsmc = lstack.enter_context(tc.tile_pool(name="vsmc", bufs=1))
            xcp = lstack.enter_context(tc.tile_pool(name="xcp", bufs=2))
            rsmp = lstack.enter_context(tc.tile_pool(name="rsmp", bufs=1))
            small = lstack.enter_context(tc.tile_pool(name="small", bufs=2))
            small1 = lstack.enter_context(tc.tile_pool(name="small1", bufs=1))
            lnb = lstack.enter_context(tc.tile_pool(name="lnb", bufs=2))
            pscore = lstack.enter_context(tc.tile_pool(name="pscore", bufs=2, space="PSUM"))
            pav = lstack.enter_context(tc.tile_pool(name="pav", bufs=2, space="PSUM"))
            pmm = lstack.enter_context(tc.tile_pool(name="pmm", bufs=2, space="PSUM"))
            pstat = lstack.enter_context(tc.tile_pool(name="pstat", bufs=1, space="PSUM"))
            if True:
                h = lpool.tile([P, DC, TPC], bf16, name="h")
                qbf = lpool.tile([P, DC, TPC], bf16, name="qbf")
                kst = lpool.tile([P, DC, TPC], bf16, name="kst")
                kpeer = lpool.tile([P, DC, TPC], bf16, name="kpeer")
                vst = lpool.tile([P, 4, VW], bf16, name="vst")
                vpeer = lpool.tile([P, 4, VW], bf16, name="vpeer")
                obf = lpool.tile([P, DC, TPC], bf16, name="obf")
                r = lpool.tile([P, 16, TPC], bf16, name="r")
                sxp = pstat.tile([1, TPC], f32, tag="sx", name="sxp")
                sqp = pstat.tile([1, TPC], f32, tag="sq", name="sqp")
                # ones columns of V_aug, set once (data writes never touch them)
                nc.vector.memset(vst[:], 1.0)

                def stats_chunk(m, first, last, eng):
                    xcb = xcp.tile([P, TPC], bf16, tag="xc", name=f"xcb_{m}")
                    eng.tensor_copy(xcb[:], x[:, m, :])
                    sqb = xcp.tile([P, TPC], bf16, tag="sq", name=f"sqb_{m}")
                    nc.vector.tensor_mul(sqb[:], xcb[:], xcb[:])
                    nc.tensor.matmul(sxp[:], ones128b[:], xcb[:], start=first, stop=last)
                    nc.tensor.matmul(sqp[:], ones128b[:], sqb[:], start=first, stop=last)

                def ln_finish(nm):
                    mu = small1.tile([1, TPC], f32, tag="mu", name=f"mu_{nm}")
                    ex2 = small1.tile([1, TPC], f32, tag="ex2", name=f"ex2_{nm}")
                    nc.vector.tensor_scalar_mul(mu[:], sxp[:], 1.0 / D)
                    nc.vector.tensor_scalar_mul(ex2[:], sqp[:], 1.0 / D)
                    var = small1.tile([1, TPC], f32, tag="var", name=f"var_{nm}")
                    nc.vector.tensor_mul(var[:], mu[:], mu[:])
                    nc.vector.tensor_sub(var[:], ex2[:], var[:])
                    nc.scalar.activation(var[:], var[:], AF.Sqrt, bias=eps_t[:], scale=1.0)
                    rstd = small1.tile([1, TPC], f32, tag="rstd", name=f"rstd_{nm}")
                    nc.vector.reciprocal(rstd[:], var[:])
                    msb2 = small1.tile([1, TPC], f32, tag="msb2", name=f"msb2_{nm}")
                    nc.vector.tensor_mul(msb2[:], mu[:], rstd[:])
                    bc1 = pmm.tile([P, TPC], f32, tag="mm", name=f"bc1_{nm}")
                    nc.tensor.matmul(bc1[:], ones1[:], rstd[:], start=True, stop=True)
                    rsb = rsmp.tile([P, TPC], f32, tag="rsb", name=f"rsb_{nm}")
                    nc.scalar.copy(rsb[:], bc1[:])
                    bc2 = pmm.tile([P, TPC], f32, tag="mm", name=f"bc2_{nm}")
                    nc.tensor.matmul(bc2[:], ones1[:], msb2[:], start=True, stop=True)
                    msb = rsmp.tile([P, TPC], f32, tag="msb", name=f"msb_{nm}")
                    nc.scalar.copy(msb[:], bc2[:])
                    return rsb, msb

                def ln_apply(out_bf, rsb, msb):
                    for hf, eng in ((0, nc.vector), (1, nc.gpsimd)):
                        sl = slice(hf * 4, hf * 4 + 4)
                        eng.tensor_mul(out_bf[:, sl, :], x[:, sl, :],
                                       rsb[:, None, :].to_broadcast([P, 4, TPC]))
                        eng.tensor_sub(out_bf[:, sl, :], out_bf[:, sl, :],
                                       msb[:, None, :].to_broadcast([P, 4, TPC]))

                # ---- initial LN1 (layer 0)
                for m in range(DC):
                    stats_chunk(m, m == 0, m == DC - 1,
                                nc.gpsimd if m % 2 else nc.vector)
                rsb0, msb0 = ln_finish("l0")
                ln_apply(h, rsb0, msb0)

                for li in range(L):
                    bqt = lnb.tile([P, DC, 1], f32, tag="bq", name=f"bqt_{li}")
                    nc.sync.dma_start(bqt[:], bq_d[li][:, :, None])
                    bkt = lnb.tile([P, DC, 1], f32, tag="bk", name=f"bkt_{li}")
                    nc.sync.dma_start(bkt[:], bk_d[li][:, :, None])

                    # ---------- K projection, stage, collective ----------
                    for hf in range(2):
                        wkh = wbig.tile([P, DC, 512], bf16, tag="w", name=f"wk_{li}_{hf}")
                        nc.sync.dma_start(wkh[:], wk[li, :, :, hf * 512:(hf + 1) * 512])
                        for mm_ in range(4):
                            m = hf * 4 + mm_
                            ps = pmm.tile([P, TPC], f32, tag="mm", name=f"kps_{li}_{m}")
                            for c in range(DC):
                                nc.tensor.matmul(ps[:], wkh[:, c, mm_ * P:(mm_ + 1) * P],
                                                 h[:, c, :], start=(c == 0), stop=(c == DC - 1))
                            nc.scalar.activation(kst[:, m, :], ps[:], AF.Identity, bias=bkt[:, m])
                    nc.sync.dma_start(
                        kvloc_k.rearrange("(p c t) -> p c t", c=DC, t=TPC), kst[:])
                    nc.gpsimd.collective_compute(
                        "AllReduce", OP.add, replica_groups=groups,
                        ins=[kvloc_k[:]], outs=[kvred_k[:]])

                    # ---------- V projection, stage, collective ----------
                    for hf in range(2):
                        wvh = wbig.tile([P, DC, 512], bf16, tag="w", name=f"wv_{li}_{hf}")
                        nc.sync.dma_start(wvh[:], wv[li, :, :, hf * 512:(hf + 1) * 512])
                        for tc4 in range(4):
                            ps = pmm.tile([P, TPC], f32, tag="mm", name=f"vps_{li}_{hf}_{tc4}")
                            for c in range(DC):
                                nc.tensor.matmul(
                                    ps[:], h[:, c, tc4 * P:(tc4 + 1) * P],
                                    wvh[:, c, :], start=(c == 0), stop=(c == DC - 1))
                            dst = vst[:, tc4, :].rearrange("p (h e) -> p h e", e=HS + 1)
                            if tc4 % 2:
                                nc.vector.tensor_copy(
                                    dst[:, hf * 8:(hf + 1) * 8, 0:HS],
                                    ps[:].rearrange("p (h e) -> p h e", e=HS))
                            else:
                                nc.scalar.copy(
                                    dst[:, hf * 8:(hf + 1) * 8, 0:HS],
                                    ps[:].rearrange("p (h e) -> p h e", e=HS))
                    nc.sync.dma_start(
                        kvloc_v.rearrange("(p c t) -> p c t", c=4, t=VW), vst[:])
                    nc.gpsimd.collective_compute(
                        "AllReduce", OP.add, replica_groups=groups,
                        ins=[kvloc_v[:]], outs=[kvred_v[:]])

                    # ---------- Q projection ----------
                    for hf in range(2):
                        wqh = wbig.tile([P, DC, 512], bf16, tag="w", name=f"wq_{li}_{hf}")
                        nc.sync.dma_start(wqh[:], wq[li, :, :, hf * 512:(hf + 1) * 512])
                        for mm_ in range(4):
                            m = hf * 4 + mm_
                            ps = pmm.tile([P, TPC], f32, tag="mm", name=f"qps_{li}_{m}")
                            for c in range(DC):
                                nc.tensor.matmul(ps[:], wqh[:, c, mm_ * P:(mm_ + 1) * P],
                                                 h[:, c, :], start=(c == 0), stop=(c == DC - 1))
                            nc.scalar.activation(qbf[:, m, :], ps[:], AF.Identity, bias=bqt[:, m])

                    # ---------- peer K/V recovery (chunked) ----------
                    for c in range(DC):
                        ks = sumc.tile([P, TPC], bf16, tag="ks", name=f"ks_{li}_{c}")
                        nc.sync.dma_start(
                            ks[:], kvred_k.rearrange("(p c t) -> p c t", c=DC, t=TPC)[:, c, :])
                        eng = nc.vector if c % 2 else nc.gpsimd
                        eng.tensor_sub(kpeer[:, c, :], ks[:], kst[:, c, :])
                    for tc4 in range(4):
                        vs = vsmc.tile([P, VW], bf16, tag="vs", name=f"vs_{li}_{tc4}")
                        nc.sync.dma_start(
                            vs[:], kvred_v.rearrange("(p c t) -> p c t", c=4, t=VW)[:, tc4, :])
                        eng = nc.vector if tc4 % 2 else nc.gpsimd
                        eng.tensor_sub(vpeer[:, tc4, :], vs[:], vst[:, tc4, :])

                    # ---------- attention ----------
                    for hd in range(H):
                        hp = (hd % 2) * HS
                        hc = hd // 2
                        sexp = sexpp.tile([P, DC, TPC], bf16, tag="sx", name=f"sexp_{li}_{hd}")
                        for kt in range(4):
                            ps = pscore.tile([P, TPC], f32, tag="sc", name=f"sL_{li}_{hd}_{kt}")
                            nc.tensor.matmul(ps[:], kst[hp:hp + HS, hc, kt * P:(kt + 1) * P],
                                             qbf[hp:hp + HS, hc, :], start=True, stop=True)
                            nc.scalar.activation(sexp[:, kt, :], ps[:], AF.Exp, scale=HS ** -0.5)
                        eng = nc.vector if hd % 2 else nc.gpsimd
                        eng.tensor_mul(sexp[:, 0:4, :], sexp[:, 0:4, :], maskl[:])
                        for kt in range(4):
                            ps = pscore.tile([P, TPC], f32, tag="sc", name=f"sR_{li}_{hd}_{kt}")
                            nc.tensor.matmul(ps[:], kpeer[hp:hp + HS, hc, kt * P:(kt + 1) * P],
                                             qbf[hp:hp + HS, hc, :], start=True, stop=True)
                            nc.scalar.activation(sexp[:, 4 + kt, :], ps[:], AF.Exp,
                                                 scale=HS ** -0.5, bias=pb[:])
                        av = pav.tile([P, TPC], f32, tag="av", name=f"av_{li}_{hd}")
                        for kt in range(4):
                            nc.tensor.matmul(av[0:HS + 1, :], vst[:, kt, hd * 65:hd * 65 + 65],
                                             sexp[:, kt, :], start=(kt == 0), stop=False)
                        for kt in range(4):
                            nc.tensor.matmul(av[0:HS + 1, :], vpeer[:, kt, hd * 65:hd * 65 + 65],
                                             sexp[:, 4 + kt, :], start=False, stop=(kt == 3))
                        rc = small.tile([1, TPC], f32, tag="rc", name=f"rc_{li}_{hd}")
                        nc.vector.reciprocal(rc[:], av[HS:HS + 1, :])
                        bc = pmm.tile([P, TPC], f32, tag="mm", name=f"bcp_{li}_{hd}")
                        nc.tensor.matmul(bc[0:HS, :], ones1[:, 0:HS], rc[:], start=True, stop=True)
                        bcs = small.tile([HS, TPC], f32, tag="bcs", name=f"bcs_{li}_{hd}")
                        nc.scalar.copy(bcs[:], bc[0:HS, :])
                        nc.vector.tensor_mul(obf[hp:hp + HS, hc, :], av[0:HS, :], bcs[:])

                    # ---------- O projection + residual + LN2 stats ----------
                    bot = lnb.tile([P, DC, 1], f32, tag="bo", name=f"bot_{li}")
                    nc.sync.dma_start(bot[:], bo_d[li][:, :, None])
                    for hf in range(2):
                        woh = wbig.tile([P, DC, 512], bf16, tag="w", name=f"wo_{li}_{hf}")
                        nc.sync.dma_start(woh[:], wo[li, :, :, hf * 512:(hf + 1) * 512])
                        for mm_ in range(4):
                            m = hf * 4 + mm_
                            ps = pmm.tile([P, TPC], f32, tag="mm", name=f"ops_{li}_{m}")
                            for c in range(DC):
                                nc.tensor.matmul(ps[:], woh[:, c, mm_ * P:(mm_ + 1) * P],
                                                 obf[:, c, :], start=(c == 0), stop=(c == DC - 1))
                            nc.vector.scalar_tensor_tensor(
                                x[:, m, :], ps[:], bot[:, m], x[:, m, :], op0=OP.add, op1=OP.add)
                            stats_chunk(m, m == 0, m == DC - 1,
                                        nc.gpsimd if m % 2 else nc.vector)
                    rsb2, msb2s = ln_finish(f"l2_{li}")
                    ln_apply(h, rsb2, msb2s)

                    # ---------- MLP (two halves of DFF) ----------
                    b1t = lnb.tile([P, FC, 1], f32, tag="b1", name=f"b1t_{li}")
                    nc.sync.dma_start(b1t[:], b1_d[li][:, :, None])
                    b2t = lnb.tile([P, DC, 1], f32, tag="b2", name=f"b2t_{li}")
                    nc.sync.dma_start(b2t[:], b2_d[li][:, :, None])
                    for fh in range(2):
                        for mfl in range(16):
                            mf = fh * 16 + mfl
                            w1t = w1p.tile([P, DC, P], bf16, tag="w1", name=f"w1_{li}_{mf}")
                            nc.sync.dma_start(w1t[:], w1[li, mf])
                            ps = pmm.tile([P, TPC], f32, tag="mm", name=f"mps_{li}_{mf}")
                            for c in range(DC):
                                nc.tensor.matmul(ps[:], w1t[:, c, :], h[:, c, :],
                                                 start=(c == 0), stop=(c == DC - 1))
                            nc.scalar.activation(r[:, mfl, :], ps[:], AF.Relu,
                                                 bias=b1t[:, mf], scale=1.0)
                        for m in range(DC):
                            w2t = w2p.tile([P, 16, P], bf16, tag="w2", name=f"w2_{li}_{fh}_{m}")
                            nc.sync.dma_start(w2t[:], w2[li, m, :, fh * 16:(fh + 1) * 16, :])
                            ps = pmm.tile([P, TPC], f32, tag="mm", name=f"m2_{li}_{fh}_{m}")
                            for f in range(16):
                                nc.tensor.matmul(ps[:], w2t[:, f, :], r[:, f, :],
                                                 start=(f == 0), stop=(f == 15))
                            if fh == 0:
                                nc.vector.scalar_tensor_tensor(
                                    x[:, m, :], ps[:], b2t[:, m], x[:, m, :],
                                    op0=OP.add, op1=OP.add)
                            else:
                                nc.vector.tensor_add(x[:, m, :], x[:, m, :], ps[:])
                                stats_chunk(m, m == 0, m == DC - 1,
                                            nc.gpsimd if m % 2 else nc.vector)
                    if li < L - 1:
                        rsb1, msb1 = ln_finish(f"l1_{li + 1}")
                        ln_apply(h, rsb1, msb1)

                # ---------- final LN ----------
                rsbf, msbf = ln_finish("lf")
                ln_apply(xf, rsbf, msbf)

            # ---------- LM head ----------
            lstack.close()
            lmstack = ExitStack()
            wg = lmstack.enter_context(tc.tile_pool(name="wg", bufs=2))
            otp = lmstack.enter_context(tc.tile_pool(name="otp", bufs=4))
            blsp = lmstack.enter_context(tc.tile_pool(name="blsp", bufs=2))
            pacc = lmstack.enter_context(tc.tile_pool(name="pacc", bufs=6, space="PSUM"))
            pbc = lmstack.enter_context(tc.tile_pool(name="pbc", bufs=2, space="PSUM"))
            if True:
                for g in range(NG):
                    g0 = g * GV
                    gn = GV
                    wgt = wg.tile([P, DC, GV * 512], bf16, tag="wg", name=f"wg_{g}")
                    nc.sync.dma_start(wgt[:], wlm[g])
                    blg = blsp.tile([1, GV * 512], f32, tag="blg", name=f"blg_{g}")
                    nc.sync.dma_start(blg[:], blm_d[None, g0 * 512:(g0 + gn) * 512])
                    blsts = []
                    for vi in range(gn):
                        bcp = pbc.tile([P, 512], f32, tag="bc", name=f"bcp_{g}_{vi}")
                        nc.tensor.matmul(bcp[:], ones1[:], blg[0:1, vi * 512:(vi + 1) * 512],
                                         start=True, stop=True)
                        blst = blsp.tile([P, 512], f32, tag=f"bls{vi}", name=f"bls_{g}_{vi}")
                        nc.scalar.activation(blst[:], bcp[:], AF.Copy)
                        blsts.append(blst)
                    for tc4 in range(4):
                        pss = [pacc.tile([P, 512], f32, tag="acc", name=f"lm_{g}_{tc4}_{vi}")
                               for vi in range(gn)]
                        for c in range(DC):
                            for vi in range(gn):
                                nc.tensor.matmul(
                                    pss[vi][:], xf[:, c, tc4 * P:(tc4 + 1) * P],
                                    wgt[:, c, vi * 512:(vi + 1) * 512],
                                    start=(c == 0), stop=(c == DC - 1))
                        for vi in range(gn):
                            vc = g0 + vi
                            if vc >= NVC:
                                continue
                            nv = min(512, V - vc * 512)
                            ott = otp.tile([P, 512], bf16, tag="ot", name=f"ot_{g}_{tc4}_{vi}")
                            nc.vector.scalar_tensor_tensor(
                                ott[:], pss[vi][:], 1.0, blsts[vi][:],
                                op0=OP.mult, op1=OP.add)
                            nc.sync.dma_start(
                                out_d[tc4 * P:(tc4 + 1) * P, vc * 512:vc * 512 + nv],
                                ott[:, 0:nv])
            lmstack.close()

    nc.compile()
    return nc


def kernel(**inputs):
    global LAST_EXEC_NS
    _install_ntff_hook()
    if "nc" not in _CACHE:
        _CACHE["nc"] = _build()
    nc = _CACHE["nc"]

    gi = {k: np.asarray(v, np.float32) if np.asarray(v).dtype == np.float32
          else np.asarray(v) for k, v in inputs.items()}
    idx = np.asarray(gi["idx"]).astype(np.int64)
    xemb = np.asarray(gi["wte"])[idx] + np.asarray(gi["wpe"])[:T][None, :, :]

    # ---- fold LN weights/biases into adjacent projections (host, fp32)
    ln1w = np.asarray(gi["ln1_w"]); ln1b = np.asarray(gi["ln1_b"])
    ln2w = np.asarray(gi["ln2_w"]); ln2b = np.asarray(gi["ln2_b"])
    lnfw = np.asarray(gi["lnf_w"]); lnfb = np.asarray(gi["lnf_b"])
    wq_e = ln1w[:, :, None] * gi["wq"]          # [L,D,D]
    wk_e = ln1w[:, :, None] * gi["wk"]
    wv_e = ln1w[:, :, None] * gi["wv"]
    bq_v = np.einsum('ld,lde->le', ln1b, gi["wq"])   # [L,D]
    bk_v = np.einsum('ld,lde->le', ln1b, gi["wk"])
    bv_v = np.einsum('ld,lde->le', ln1b, gi["wv"])
    bo_e = gi["bo"] + np.einsum('ld,lde->le', bv_v, gi["wo"])
    w1_e = ln2w[:, :, None] * gi["w1"]
    b1_e = gi["b1"] + np.einsum('ld,lde->le', ln2b, gi["w1"])
    wlm_e = lnfw[:, None] * gi["wlm"]
    blm_e = gi["blm"] + lnfb @ gi["wlm"]

    def pack_sq(w):   # [L, 1024, N] -> [L, 128, 8, N]
        Lw, Kw, Nw = w.shape
        return np.ascontiguousarray(
            w.reshape(Lw, DC, P, Nw).transpose(0, 2, 1, 3).astype(ml_dtypes.bfloat16))

    w1p = np.ascontiguousarray(
        w1_e.reshape(L, DC, P, FC, P).transpose(0, 3, 2, 1, 4).astype(ml_dtypes.bfloat16))
    w2p = np.ascontiguousarray(
        np.asarray(gi["w2"]).reshape(L, FC, P, DC, P).transpose(0, 3, 2, 1, 4)
        .astype(ml_dtypes.bfloat16))
    wlmp = np.zeros((D, VPAD), np.float32)
    wlmp[:, :V] = wlm_e
    wlmp = np.ascontiguousarray(
        wlmp.reshape(DC, P, NG, GV * 512).transpose(2, 1, 0, 3).astype(ml_dtypes.bfloat16))
    blmp = np.zeros((VPAD,), np.float32)
    blmp[:V] = blm_e

    def packv(v):  # [.., N] -> [.., P, N//P]
        v = np.asarray(v, np.float32)
        nch = v.shape[-1] // P
        return np.ascontiguousarray(
            v.reshape(v.shape[:-1] + (nch, P)).swapaxes(-1, -2))

    # local diagonal causal mask: same on every core
    ml_m = np.zeros((P, 4, TPC), np.float32)
    k_rel = np.arange(P)[:, None] + (np.arange(4) * P)[None, :]
    ml_m[:] = (k_rel[:, :, None] <= np.arange(TPC)[None, None, :])

    shared = dict(
        wq=pack_sq(wq_e), wk=pack_sq(wk_e), wv=pack_sq(wv_e),
        wo=pack_sq(np.asarray(gi["wo"], np.float32)),
        w1=w1p, w2=w2p, wlm=wlmp,
        bq=packv(bq_v), bk=packv(bk_v), bo=packv(bo_e),
        b1=packv(b1_e), b2=packv(np.asarray(gi["b2"], np.float32)),
        blm=np.ascontiguousarray(blmp),
        maskl=ml_m.astype(ml_dtypes.bfloat16),
    )

    in_maps = []
    for c in range(8):
        b, half = c // 2, c % 2
        sl = slice(half * TPC, (half + 1) * TPC)
        im = dict(shared)
        im["xembT"] = np.ascontiguousarray(xemb[b, sl].T, dtype=np.float32)
        im["pbias"] = np.full((P, 1), 0.0 if half else -60000.0, np.float32)
        in_maps.append(im)

    res = run_bass_kernel_spmd(nc, in_maps, list(range(8)),
                               trace=bool(os.environ.get("BASS_TRACE")))
    LAST_EXEC_NS = res.exec_time_ns

    out = np.empty((B, T, V), np.float32)
    for c in range(8):
        b, half = c // 2, c % 2
        out[b, half * TPC:(half + 1) * TPC] = res.results[c]["out"].astype(np.float32)
    return out


# revision 10
# speedup vs baseline: 1.2068x; 1.0013x over previous
"""GPT-2 (L=8, D=1024, H=16, V=50257, B=4, T=1024) forward on 8 TRN2 NeuronCores.

Sharding: core c handles batch b=c//2, sequence half h=c%2 (512 tokens).
Weights replicated (bf16). Per layer, K/V are exchanged between the two cores
of a batch-pair with an AllReduce(add); each core recovers the peer half by
subtracting its own contribution (bf16 sub). Attention chunk order is
core-relative: chunks 0-3 = local keys (direct from SBUF, no collective wait),
chunks 4-7 = peer keys. Causality is data-driven: a diagonal [128,4,512] mask
(identical on all cores) for the local half, and a per-core exp bias
(0 or -60000) that zeroes the whole peer half on first-half cores.

LN weights/biases are folded into the adjacent projection weights host-side,
so on-chip LN is a pure (x-mu)*rstd; stats are accumulated chunk-by-chunk as
the residual stream is produced. LM head runs in vocab groups of 6 sharing
the stationary activations across 6 PSUM banks, bf16 output (host upcasts).
"""

import os
import sys
import types
from contextlib import ExitStack

import numpy as np
import ml_dtypes

import concourse.bass as bass
import concourse.mybir as mybir
import concourse.tile as tile
from concourse import bacc
from concourse.bass_utils import run_bass_kernel_spmd

f32 = mybir.dt.float32
bf16 = mybir.dt.bfloat16
AF = mybir.ActivationFunctionType
OP = mybir.AluOpType

L, D, H, V, DFF = 8, 1024, 16, 50257, 4096
HS = D // H          # 64
B, T = 4, 1024
TPC = 512            # tokens per core
P = 128
DC = D // P          # 8 d-chunks
FC = DFF // P        # 32 dff-chunks
NVC = (V + 511) // 512   # 99 vocab chunks
GV = 6               # lm-head vocab chunks per group
NG = (NVC + GV - 1) // GV        # 17 groups
NVC2 = NG * GV                   # 102 (padded)
VPAD = NVC2 * 512
EPS = 1e-5
VW = H * (HS + 1)    # 1040

K_SZ = P * DC * TPC           # 524288
V_SZ = P * 4 * VW             # 532480

LAST_EXEC_NS = None
_CACHE = {}


def _install_ntff_hook():
    """Provide antenv.axon_hooks if the image lacks it, so trace=True works."""
    try:
        import antenv
        try:
            from antenv import axon_hooks  # noqa: F401
            return
        except ImportError:
            pass
        hooks_mod = types.ModuleType("antenv.axon_hooks")
        _hook = [None]
        hooks_mod.set_axon_ntff_profile_hook = lambda h: _hook.__setitem__(0, h)
        hooks_mod.get_axon_ntff_profile_hook = lambda: _hook[0]
        sys.modules["antenv.axon_hooks"] = hooks_mod
        antenv.axon_hooks = hooks_mod
        from trn_agent_boot.trn_boot import _ntff_profile_via_ctypes
        hooks_mod.set_axon_ntff_profile_hook(
            _ntff_profile_via_ctypes("/opt/axon/libaxon_pjrt.so"))
    except Exception:
        pass


def _build():
    nc = bacc.Bacc(None, target_bir_lowering=False, debug=False)

    xembT = nc.dram_tensor("xembT", [D, TPC], f32, kind="ExternalInput")
    wq = nc.dram_tensor("wq", [L, P, DC, D], bf16, kind="ExternalInput")
    wk = nc.dram_tensor("wk", [L, P, DC, D], bf16, kind="ExternalInput")
    wv = nc.dram_tensor("wv", [L, P, DC, D], bf16, kind="ExternalInput")
    wo = nc.dram_tensor("wo", [L, P, DC, D], bf16, kind="ExternalInput")
    w1 = nc.dram_tensor("w1", [L, FC, P, DC, P], bf16, kind="ExternalInput")
    w2 = nc.dram_tensor("w2", [L, DC, P, FC, P], bf16, kind="ExternalInput")
    wlm = nc.dram_tensor("wlm", [NG, P, DC, GV * 512], bf16, kind="ExternalInput")
    bq_d = nc.dram_tensor("bq", [L, P, DC], f32, kind="ExternalInput")
    bk_d = nc.dram_tensor("bk", [L, P, DC], f32, kind="ExternalInput")
    bo_d = nc.dram_tensor("bo", [L, P, DC], f32, kind="ExternalInput")
    b1_d = nc.dram_tensor("b1", [L, P, FC], f32, kind="ExternalInput")
    b2_d = nc.dram_tensor("b2", [L, P, DC], f32, kind="ExternalInput")
    blm_d = nc.dram_tensor("blm", [VPAD], f32, kind="ExternalInput")
    maskl_d = nc.dram_tensor("maskl", [P, P], bf16, kind="ExternalInput")
    pbias_d = nc.dram_tensor("pbias", [P, 1], f32, kind="ExternalInput")
    out_d = nc.dram_tensor("out", [TPC, V], bf16, kind="ExternalOutput")

    kvloc_k = nc.dram_tensor("kvloc_k", [K_SZ], bf16)
    kvred_k = nc.dram_tensor("kvred_k", [K_SZ], bf16)
    kvloc_v = nc.dram_tensor("kvloc_v", [V_SZ], bf16)
    kvred_v = nc.dram_tensor("kvred_v", [V_SZ], bf16)
    groups = [[0, 1], [2, 3], [4, 5], [6, 7]]

    with tile.TileContext(nc) as tc:
        with (
            tc.tile_pool(name="cpool", bufs=1) as cpool,
            tc.tile_pool(name="csm", bufs=2) as csm,
        ):
            # ---- persistent / common tiles
            x = cpool.tile([P, DC, TPC], f32, name="x")
            xf = cpool.tile([P, DC, TPC], bf16, name="xf")
            maskl = cpool.tile([P, P], bf16, name="maskl")
            pb = cpool.tile([P, 1], f32, name="pb")
            ones1 = cpool.tile([1, P], f32, name="ones1")
            ones128b = cpool.tile([P, 1], bf16, name="ones128b")
            eps_t = cpool.tile([1, 1], f32, name="eps_t")
            nc.vector.memset(ones1[:], 1.0)
            nc.vector.memset(ones128b[:], 1.0)
            nc.vector.memset(eps_t[:], EPS)
            nc.sync.dma_start(maskl[:], maskl_d[:])
            nc.sync.dma_start(pb[:], pbias_d[:])
            nc.sync.dma_start(x[:], xembT.rearrange("(c p) t -> p c t", p=P))

            lstack = ExitStack()
            lpool = lstack.enter_context(tc.tile_pool(name="lpool", bufs=1))
            wbig = lstack.enter_context(tc.tile_pool(name="wbig", bufs=2))
            w1p = lstack.enter_context(tc.tile_pool(name="w1p", bufs=3))
            w2p = lstack.enter_context(tc.tile_pool(name="w2p", bufs=2))
            sexpp = lstack.enter_context(tc.tile_pool(name="sexpp", bufs=4))
            sumc = lstack.enter_context(tc.tile_pool(name="sumc", bufs=2))
            vsmc = lstack.enter_context(tc.tile_pool(name="vsmc", bufs=1))
            xcp = lstack.enter_context(tc.tile_pool(name="xcp", bufs=2))
            rsmp = lstack.enter_context(tc.tile_pool(name="rsmp", bufs=1))
            small = lstack.enter_context(tc.tile_pool(name="small", bufs=2))
            small1 = lstack.enter_context(tc.tile_pool(name="small1", bufs=1))
            lnb = lstack.enter_context(tc.tile_pool(name="lnb", bufs=2))
            pscore = lstack.enter_context(tc.tile_pool(name="pscore", bufs=2, space="PSUM"))
            pav = lstack.enter_context(tc.tile_pool(name="pav", bufs=2, space="PSUM"))
            pmm = lstack.enter_context(tc.tile_pool(name="pmm", bufs=2, space="PSUM"))
            pstat = lstack.enter_context(tc.tile_pool(name="pstat", bufs=1, space="PSUM"))
            if True:
                h = lpool.tile([P, DC, TPC], bf16, name="h")
                qbf = lpool.tile([P, DC, TPC], bf16, name="qbf")
                kst = lpool.tile([P, DC, TPC], bf16, name="kst")
                kpeer = lpool.tile([P, DC, TPC], bf16, name="kpeer")
                vst = lpool.tile([P, 4, VW], bf16, name="vst")
                vpeer = lpool.tile([P, 4, VW], bf16, name="vpeer")
                obf = lpool.tile([P, DC, TPC], bf16, name="obf")
                xbf = lpool.tile([P, DC, TPC], bf16, name="xbf")
                r = lpool.tile([P, 16, TPC], bf16, name="r")
                sxp = pstat.tile([1, TPC], f32, tag="sx", name="sxp")
                sqp = pstat.tile([1, TPC], f32, tag="sq", name="sqp")
                # ones columns of V_aug, set once (data writes never touch them)
                nc.vector.memset(vst[:], 1.0)

                def stats_chunk(m, first, last, eng):
                    eng.tensor_copy(xbf[:, m, :], x[:, m, :])
                    sqb = xcp.tile([P, TPC], bf16, tag="sq", name=f"sqb_{m}")
                    nc.vector.tensor_mul(sqb[:], xbf[:, m, :], xbf[:, m, :])
                    nc.tensor.matmul(sxp[:], ones128b[:], xbf[:, m, :], start=first, stop=last)
                    nc.tensor.matmul(sqp[:], ones128b[:], sqb[:], start=first, stop=last)

                def ln_finish(nm):
                    mu = small1.tile([1, TPC], f32, tag="mu", name=f"mu_{nm}")
                    ex2 = small1.tile([1, TPC], f32, tag="ex2", name=f"ex2_{nm}")
                    nc.vector.tensor_scalar_mul(mu[:], sxp[:], 1.0 / D)
                    nc.vector.tensor_scalar_mul(ex2[:], sqp[:], 1.0 / D)
                    var = small1.tile([1, TPC], f32, tag="var", name=f"var_{nm}")
                    nc.vector.tensor_mul(var[:], mu[:], mu[:])
                    nc.vector.tensor_sub(var[:], ex2[:], var[:])
                    nc.scalar.activation(var[:], var[:], AF.Sqrt, bias=eps_t[:], scale=1.0)
                    rstd = small1.tile([1, TPC], f32, tag="rstd", name=f"rstd_{nm}")
                    nc.vector.reciprocal(rstd[:], var[:])
                    msb2 = small1.tile([1, TPC], f32, tag="msb2", name=f"msb2_{nm}")
                    nc.vector.tensor_mul(msb2[:], mu[:], rstd[:])
                    bc1 = pmm.tile([P, TPC], f32, tag="mm", name=f"bc1_{nm}")
                    nc.tensor.matmul(bc1[:], ones1[:], rstd[:], start=True, stop=True)
                    rsb = rsmp.tile([P, TPC], bf16, tag="rsb", name=f"rsb_{nm}")
                    nc.scalar.copy(rsb[:], bc1[:])
                    bc2 = pmm.tile([P, TPC], f32, tag="mm", name=f"bc2_{nm}")
                    nc.tensor.matmul(bc2[:], ones1[:], msb2[:], start=True, stop=True)
                    msb = rsmp.tile([P, TPC], bf16, tag="msb", name=f"msb_{nm}")
                    nc.scalar.copy(msb[:], bc2[:])
                    return rsb, msb

                def ln_apply(out_bf, rsb, msb):
                    for hf in range(2):
                        sl = slice(hf * 4, hf * 4 + 4)
                        nc.vector.tensor_mul(out_bf[:, sl, :], xbf[:, sl, :],
                                             rsb[:, None, :].to_broadcast([P, 4, TPC]))
                        nc.vector.tensor_sub(out_bf[:, sl, :], out_bf[:, sl, :],
                                             msb[:, None, :].to_broadcast([P, 4, TPC]))

                # ---- initial LN1 (layer 0)
                for m in range(DC):
                    stats_chunk(m, m == 0, m == DC - 1,
                                nc.gpsimd if m % 2 else nc.vector)
                rsb0, msb0 = ln_finish("l0")
                ln_apply(h, rsb0, msb0)

                for li in range(L):
                    bqt = lnb.tile([P, DC, 1], f32, tag="bq", name=f"bqt_{li}")
                    nc.sync.dma_start(bqt[:], bq_d[li][:, :, None])
                    bkt = lnb.tile([P, DC, 1], f32, tag="bk", name=f"bkt_{li}")
                    nc.sync.dma_start(bkt[:], bk_d[li][:, :, None])

                    # ---------- K projection, stage, collective ----------
                    for hf in range(2):
                        wkh = wbig.tile([P, DC, 512], bf16, tag="w", name=f"wk_{li}_{hf}")
                        nc.scalar.dma_start(wkh[:], wk[li, :, :, hf * 512:(hf + 1) * 512])
                        for mm_ in range(4):
                            m = hf * 4 + mm_
                            ps = pmm.tile([P, TPC], f32, tag="mm", name=f"kps_{li}_{m}")
                            for c in range(DC):
                                nc.tensor.matmul(ps[:], wkh[:, c, mm_ * P:(mm_ + 1) * P],
                                                 h[:, c, :], start=(c == 0), stop=(c == DC - 1))
                            nc.scalar.activation(kst[:, m, :], ps[:], AF.Identity, bias=bkt[:, m])
                    nc.sync.dma_start(
                        kvloc_k.rearrange("(p c t) -> p c t", c=DC, t=TPC), kst[:])
                    nc.gpsimd.collective_compute(
                        "AllReduce", OP.add, replica_groups=groups,
                        ins=[kvloc_k[:]], outs=[kvred_k[:]])

                    # ---------- V projection, stage, collective ----------
                    for hf in range(2):
                        wvh = wbig.tile([P, DC, 512], bf16, tag="w", name=f"wv_{li}_{hf}")
                        nc.scalar.dma_start(wvh[:], wv[li, :, :, hf * 512:(hf + 1) * 512])
                        for tc4 in range(4):
                            ps = pmm.tile([P, TPC], f32, tag="mm", name=f"vps_{li}_{hf}_{tc4}")
                            for c in range(DC):
                                nc.tensor.matmul(
                                    ps[:], h[:, c, tc4 * P:(tc4 + 1) * P],
                                    wvh[:, c, :], start=(c == 0), stop=(c == DC - 1))
                            dst = vst[:, tc4, :].rearrange("p (h e) -> p h e", e=HS + 1)
                            if tc4 % 2:
                                nc.vector.tensor_copy(
                                    dst[:, hf * 8:(hf + 1) * 8, 0:HS],
                                    ps[:].rearrange("p (h e) -> p h e", e=HS))
                            else:
                                nc.scalar.copy(
                                    dst[:, hf * 8:(hf + 1) * 8, 0:HS],
                                    ps[:].rearrange("p (h e) -> p h e", e=HS))
                    nc.sync.dma_start(
                        kvloc_v.rearrange("(p c t) -> p c t", c=4, t=VW), vst[:])
                    nc.gpsimd.collective_compute(
                        "AllReduce", OP.add, replica_groups=groups,
                        ins=[kvloc_v[:]], outs=[kvred_v[:]])

                    # ---------- Q projection ----------
                    for hf in range(2):
                        wqh = wbig.tile([P, DC, 512], bf16, tag="w", name=f"wq_{li}_{hf}")
                        nc.scalar.dma_start(wqh[:], wq[li, :, :, hf * 512:(hf + 1) * 512])
                        for mm_ in range(4):
                            m = hf * 4 + mm_
                            ps = pmm.tile([P, TPC], f32, tag="mm", name=f"qps_{li}_{m}")
                            for c in range(DC):
                                nc.tensor.matmul(ps[:], wqh[:, c, mm_ * P:(mm_ + 1) * P],
                                                 h[:, c, :], start=(c == 0), stop=(c == DC - 1))
                            nc.scalar.activation(qbf[:, m, :], ps[:], AF.Identity, bias=bqt[:, m])

                    # ---------- peer K/V recovery (chunked) ----------
                    for c in range(DC):
                        ks = sumc.tile([P, TPC], bf16, tag="ks", name=f"ks_{li}_{c}")
                        nc.sync.dma_start(
                            ks[:], kvred_k.rearrange("(p c t) -> p c t", c=DC, t=TPC)[:, c, :])
                        nc.vector.tensor_sub(kpeer[:, c, :], ks[:], kst[:, c, :])
                    for tc4 in range(4):
                        vs = vsmc.tile([P, VW], bf16, tag="vs", name=f"vs_{li}_{tc4}")
                        nc.sync.dma_start(
                            vs[:], kvred_v.rearrange("(p c t) -> p c t", c=4, t=VW)[:, tc4, :])
                        nc.vector.tensor_sub(vpeer[:, tc4, :], vs[:], vst[:, tc4, :])

                    # ---------- attention ----------
                    for hd in range(H):
                        hp = (hd % 2) * HS
                        hc = hd // 2
                        sexp = sexpp.tile([P, DC, TPC], bf16, tag="sx", name=f"sexp_{li}_{hd}")
                        for kt in range(4):
                            nq = TPC - kt * P
                            ps = pscore.tile([P, TPC], f32, tag="sc", name=f"sL_{li}_{hd}_{kt}")
                            nc.tensor.matmul(ps[:, 0:nq], kst[hp:hp + HS, hc, kt * P:(kt + 1) * P],
                                             qbf[hp:hp + HS, hc, kt * P:], start=True, stop=True)
                            nc.scalar.activation(sexp[:, kt, kt * P:], ps[:, 0:nq],
                                                 AF.Exp, scale=HS ** -0.5)
                            nc.vector.tensor_mul(sexp[:, kt, kt * P:(kt + 1) * P],
                                                 sexp[:, kt, kt * P:(kt + 1) * P], maskl[:])
                        for kt in range(4):
                            ps = pscore.tile([P, TPC], f32, tag="sc", name=f"sR_{li}_{hd}_{kt}")
                            nc.tensor.matmul(ps[:], kpeer[hp:hp + HS, hc, kt * P:(kt + 1) * P],
                                             qbf[hp:hp + HS, hc, :], start=True, stop=True)
                            nc.scalar.activation(sexp[:, 4 + kt, :], ps[:], AF.Exp,
                                                 scale=HS ** -0.5, bias=pb[:])
                        av = pav.tile([P, TPC], f32, tag="av", name=f"av_{li}_{hd}")
                        for kt in range(4):
                            nq = TPC - kt * P
                            nc.tensor.matmul(av[0:HS + 1, kt * P:], vst[:, kt, hd * 65:hd * 65 + 65],
                                             sexp[:, kt, kt * P:], start=(kt == 0), stop=False,
                                             skip_group_check=True)
                        for kt in range(4):
                            nc.tensor.matmul(av[0:HS + 1, :], vpeer[:, kt, hd * 65:hd * 65 + 65],
                                             sexp[:, 4 + kt, :], start=False, stop=(kt == 3),
                                             skip_group_check=True)
                        rc = small.tile([1, TPC], f32, tag="rc", name=f"rc_{li}_{hd}")
                        nc.vector.reciprocal(rc[:], av[HS:HS + 1, :])
                        bc = pmm.tile([P, TPC], f32, tag="mm", name=f"bcp_{li}_{hd}")
                        nc.tensor.matmul(bc[0:HS, :], ones1[:, 0:HS], rc[:], start=True, stop=True)
                        bcs = small.tile([HS, TPC], f32, tag="bcs", name=f"bcs_{li}_{hd}")
                        nc.vector.tensor_copy(bcs[:], bc[0:HS, :])
                        nc.vector.tensor_mul(obf[hp:hp + HS, hc, :], av[0:HS, :], bcs[:])

                    # ---------- O projection + residual + LN2 stats ----------
                    bot = lnb.tile([P, DC, 1], f32, tag="bo", name=f"bot_{li}")
                    nc.sync.dma_start(bot[:], bo_d[li][:, :, None])
                    for hf in range(2):
                        woh = wbig.tile([P, DC, 512], bf16, tag="w", name=f"wo_{li}_{hf}")
                        nc.scalar.dma_start(woh[:], wo[li, :, :, hf * 512:(hf + 1) * 512])
                        for mm_ in range(4):
                            m = hf * 4 + mm_
                            ps = pmm.tile([P, TPC], f32, tag="mm", name=f"ops_{li}_{m}")
                            for c in range(DC):
                                nc.tensor.matmul(ps[:], woh[:, c, mm_ * P:(mm_ + 1) * P],
                                                 obf[:, c, :], start=(c == 0), stop=(c == DC - 1))
                            nc.vector.scalar_tensor_tensor(
                                x[:, m, :], ps[:], bot[:, m], x[:, m, :], op0=OP.add, op1=OP.add)
                            stats_chunk(m, m == 0, m == DC - 1,
                                        nc.gpsimd if m % 2 else nc.vector)
                    rsb2, msb2s = ln_finish(f"l2_{li}")
                    ln_apply(h, rsb2, msb2s)

                    # ---------- MLP (two halves of DFF) ----------
                    b1t = lnb.tile([P, FC, 1], f32, tag="b1", name=f"b1t_{li}")
                    nc.sync.dma_start(b1t[:], b1_d[li][:, :, None])
                    b2t = lnb.tile([P, DC, 1], f32, tag="b2", name=f"b2t_{li}")
                    nc.sync.dma_start(b2t[:], b2_d[li][:, :, None])
                    for fh in range(2):
                        for mfl in range(16):
                            mf = fh * 16 + mfl
                            w1t = w1p.tile([P, DC, P], bf16, tag="w1", name=f"w1_{li}_{mf}")
                            nc.scalar.dma_start(w1t[:], w1[li, mf])
                            ps = pmm.tile([P, TPC], f32, tag="mm", name=f"mps_{li}_{mf}")
                            for c in range(DC):
                                nc.tensor.matmul(ps[:], w1t[:, c, :], h[:, c, :],
                                                 start=(c == 0), stop=(c == DC - 1))
                            nc.scalar.activation(r[:, mfl, :], ps[:], AF.Relu,
                                                 bias=b1t[:, mf], scale=1.0)
                        for m in range(DC):
                            w2t = w2p.tile([P, 16, P], bf16, tag="w2", name=f"w2_{li}_{fh}_{m}")
                            nc.scalar.dma_start(w2t[:], w2[li, m, :, fh * 16:(fh + 1) * 16, :])
                            ps = pmm.tile([P, TPC], f32, tag="mm", name=f"m2_{li}_{fh}_{m}")
                            for f in range(16):
                                nc.tensor.matmul(ps[:], w2t[:, f, :], r[:, f, :],
                                                 start=(f == 0), stop=(f == 15))
                            if fh == 0:
                                nc.vector.scalar_tensor_tensor(
                                    x[:, m, :], ps[:], b2t[:, m], x[:, m, :],
                                    op0=OP.add, op1=OP.add)
                            else:
                                nc.vector.tensor_add(x[:, m, :], x[:, m, :], ps[:])
                                stats_chunk(m, m == 0, m == DC - 1,
                                            nc.gpsimd if m % 2 else nc.vector)
                    if li < L - 1:
                        rsb1, msb1 = ln_finish(f"l1_{li + 1}")
                        ln_apply(h, rsb1, msb1)

                # ---------- final LN ----------
                rsbf, msbf = ln_finish("lf")
                ln_apply(xf, rsbf, msbf)

            # ---------- LM head ----------
            lstack.close()
            lmstack = ExitStack()
            wg = lmstack.enter_context(tc.tile_pool(name="wg", bufs=2))
            otp = lmstack.enter_context(tc.tile_pool(name="otp", bufs=4))
            blsp = lmstack.enter_context(tc.tile_pool(name="blsp", bufs=2))
            pacc = lmstack.enter_context(tc.tile_pool(name="pacc", bufs=6, space="PSUM"))
            pbc = lmstack.enter_context(tc.tile_pool(name="pbc", bufs=2, space="PSUM"))
            if True:
                for g in range(NG):
                    g0 = g * GV
                    gn = GV
                    wgt = wg.tile([P, DC, GV * 512], bf16, tag="wg", name=f"wg_{g}")
                    nc.scalar.dma_start(wgt[:], wlm[g])
                    blg = blsp.tile([1, GV * 512], f32, tag="blg", name=f"blg_{g}")
                    nc.sync.dma_start(blg[:], blm_d[None, g0 * 512:(g0 + gn) * 512])
                    blsts = []
                    for vi in range(gn):
                        bcp = pbc.tile([P, 512], f32, tag="bc", name=f"bcp_{g}_{vi}")
                        nc.tensor.matmul(bcp[:], ones1[:], blg[0:1, vi * 512:(vi + 1) * 512],
                                         start=True, stop=True)
                        blst = blsp.tile([P, 512], f32, tag=f"bls{vi}", name=f"bls_{g}_{vi}")
                        nc.scalar.activation(blst[:], bcp[:], AF.Copy)
                        blsts.append(blst)
                    for tc4 in range(4):
                        pss = [pacc.tile([P, 512], f32, tag="acc", name=f"lm_{g}_{tc4}_{vi}")
                               for vi in range(gn)]
                        for c in range(DC):
                            for vi in range(gn):
                                nc.tensor.matmul(
                                    pss[vi][:], xf[:, c, tc4 * P:(tc4 + 1) * P],
                                    wgt[:, c, vi * 512:(vi + 1) * 512],
                                    start=(c == 0), stop=(c == DC - 1))
                        for vi in range(gn):
                            vc = g0 + vi
                            if vc >= NVC:
                                continue
                            nv = min(512, V - vc * 512)
                            ott = otp.tile([P, 512], bf16, tag="ot", name=f"ot_{g}_{tc4}_{vi}")
                            nc.vector.scalar_tensor_tensor(
                                ott[:], pss[vi][:], 1.0, blsts[vi][:],
                                op0=OP.mult, op1=OP.add)
                            nc.sync.dma_start(
                                out_d[tc4 * P:(tc4 + 1) * P, vc * 512:vc * 512 + nv],
                                ott[:, 0:nv])
            lmstack.close()

    nc.compile()
    return nc


def kernel(**inputs):
    global LAST_EXEC_NS
    _install_ntff_hook()
    if "nc" not in _CACHE:
        _CACHE["nc"] = _build()
    nc = _CACHE["nc"]

    gi = {k: np.asarray(v, np.float32) if np.asarray(v).dtype == np.float32
          else np.asarray(v) for k, v in inputs.items()}
    idx = np.asarray(gi["idx"]).astype(np.int64)
    xemb = np.asarray(gi["wte"])[idx] + np.asarray(gi["wpe"])[:T][None, :, :]

    # ---- fold LN weights/biases into adjacent projections (host, fp32)
    ln1w = np.asarray(gi["ln1_w"]); ln1b = np.asarray(gi["ln1_b"])
    ln2w = np.asarray(gi["ln2_w"]); ln2b = np.asarray(gi["ln2_b"])
    lnfw = np.asarray(gi["lnf_w"]); lnfb = np.asarray(gi["lnf_b"])
    wq_e = ln1w[:, :, None] * gi["wq"]          # [L,D,D]
    wk_e = ln1w[:, :, None] * gi["wk"]
    wv_e = ln1w[:, :, None] * gi["wv"]
    bq_v = np.einsum('ld,lde->le', ln1b, gi["wq"])   # [L,D]
    bk_v = np.einsum('ld,lde->le', ln1b, gi["wk"])
    bv_v = np.einsum('ld,lde->le', ln1b, gi["wv"])
    bo_e = gi["bo"] + np.einsum('ld,lde->le', bv_v, gi["wo"])
    w1_e = ln2w[:, :, None] * gi["w1"]
    b1_e = gi["b1"] + np.einsum('ld,lde->le', ln2b, gi["w1"])
    wlm_e = lnfw[:, None] * gi["wlm"]
    blm_e = gi["blm"] + lnfb @ gi["wlm"]

    def pack_sq(w):   # [L, 1024, N] -> [L, 128, 8, N]
        Lw, Kw, Nw = w.shape
        return np.ascontiguousarray(
            w.reshape(Lw, DC, P, Nw).transpose(0, 2, 1, 3).astype(ml_dtypes.bfloat16))

    w1p = np.ascontiguousarray(
        w1_e.reshape(L, DC, P, FC, P).transpose(0, 3, 2, 1, 4).astype(ml_dtypes.bfloat16))
    w2p = np.ascontiguousarray(
        np.asarray(gi["w2"]).reshape(L, FC, P, DC, P).transpose(0, 3, 2, 1, 4)
        .astype(ml_dtypes.bfloat16))
    wlmp = np.zeros((D, VPAD), np.float32)
    wlmp[:, :V] = wlm_e
    wlmp = np.ascontiguousarray(
        wlmp.reshape(DC, P, NG, GV * 512).transpose(2, 1, 0, 3).astype(ml_dtypes.bfloat16))
    blmp = np.zeros((VPAD,), np.float32)
    blmp[:V] = blm_e

    def packv(v):  # [.., N] -> [.., P, N//P]
        v = np.asarray(v, np.float32)
        nch = v.shape[-1] // P
        return np.ascontiguousarray(
            v.reshape(v.shape[:-1] + (nch, P)).swapaxes(-1, -2))

    # diagonal-block causal triangle: same on every core and every chunk
    ml_m = (np.arange(P)[:, None] <= np.arange(P)[None, :]).astype(np.float32)

    shared = dict(
        wq=pack_sq(wq_e), wk=pack_sq(wk_e), wv=pack_sq(wv_e),
        wo=pack_sq(np.asarray(gi["wo"], np.float32)),
        w1=w1p, w2=w2p, wlm=wlmp,
        bq=packv(bq_v), bk=packv(bk_v), bo=packv(bo_e),
        b1=packv(b1_e), b2=packv(np.asarray(gi["b2"], np.float32)),
        blm=np.ascontiguousarray(blmp),
        maskl=ml_m.astype(ml_dtypes.bfloat16),
    )

    in_maps = []
    for c in range(8):
        b, half = c // 2, c % 2
        sl = slice(half * TPC, (half + 1) * TPC)
        im = dict(shared)
        im["xembT"] = np.ascontiguousarray(xemb[b, sl].T, dtype=np.float32)
        im["pbias"] = np.full((P, 1), 0.0 if half else -60000.0, np.float32)
        in_maps.append(im)

    res = run_bass_kernel_spmd(nc, in_maps, list(range(8)),
                               trace=bool(os.environ.get("BASS_TRACE")))
    LAST_EXEC_NS = res.exec_time_ns

    out = np.empty((B, T, V), np.float32)
    for c in range(8):
        b, half = c // 2, c % 2
        out[b, half * TPC:(half + 1) * TPC] = res.results[c]["out"].astype(np.float32)
    return out


# revision 13
# speedup vs baseline: 1.2572x; 1.0417x over previous
"""GPT-2 (L=8, D=1024, H=16, V=50257, B=4, T=1024) forward on 8 TRN2 NeuronCores.

Sharding: core c handles batch b=c//2, sequence half h=c%2 (512 tokens).
Weights replicated (bf16). Per layer, K/V are exchanged between the two cores
of a batch-pair with an AllReduce(add); each core recovers the peer half by
subtracting its own contribution (bf16 sub). Attention chunk order is
core-relative: chunks 0-3 = local keys (direct from SBUF, no collective wait),
chunks 4-7 = peer keys. Causality is data-driven: a diagonal [128,4,512] mask
(identical on all cores) for the local half, and a per-core exp bias
(0 or -60000) that zeroes the whole peer half on first-half cores.

LN weights/biases are folded into the adjacent projection weights host-side,
so on-chip LN is a pure (x-mu)*rstd; stats are accumulated chunk-by-chunk as
the residual stream is produced. LM head runs in vocab groups of 6 sharing
the stationary activations across 6 PSUM banks, bf16 output (host upcasts).
"""

import os
import sys
import types
from contextlib import ExitStack

import numpy as np
import ml_dtypes

import concourse.bass as bass
import concourse.mybir as mybir
import concourse.tile as tile
from concourse import bacc
from concourse.bass_utils import run_bass_kernel_spmd

f32 = mybir.dt.float32
bf16 = mybir.dt.bfloat16
AF = mybir.ActivationFunctionType
OP = mybir.AluOpType

L, D, H, V, DFF = 8, 1024, 16, 50257, 4096
HS = D // H          # 64
B, T = 4, 1024
TPC = 512            # tokens per core
P = 128
DC = D // P          # 8 d-chunks
FC = DFF // P        # 32 dff-chunks
NVC = (V + 511) // 512   # 99 vocab chunks
GV = 6               # lm-head vocab chunks per group
NG = (NVC + GV - 1) // GV        # 17 groups
NVC2 = NG * GV                   # 102 (padded)
VPAD = NVC2 * 512
EPS = 1e-5
VW = H * (HS + 1)    # 1040

K_SZ = P * DC * TPC           # 524288
V_SZ = P * 4 * VW             # 532480

LAST_EXEC_NS = None
_CACHE = {}


def _install_ntff_hook():
    """Provide antenv.axon_hooks if the image lacks it, so trace=True works."""
    try:
        import antenv
        try:
            from antenv import axon_hooks  # noqa: F401
            return
        except ImportError:
            pass
        hooks_mod = types.ModuleType("antenv.axon_hooks")
        _hook = [None]
        hooks_mod.set_axon_ntff_profile_hook = lambda h: _hook.__setitem__(0, h)
        hooks_mod.get_axon_ntff_profile_hook = lambda: _hook[0]
        sys.modules["antenv.axon_hooks"] = hooks_mod
        antenv.axon_hooks = hooks_mod
        from trn_agent_boot.trn_boot import _ntff_profile_via_ctypes
        hooks_mod.set_axon_ntff_profile_hook(
            _ntff_profile_via_ctypes("/opt/axon/libaxon_pjrt.so"))
    except Exception:
        pass


def _build():
    nc = bacc.Bacc(None, target_bir_lowering=False, debug=False)

    xembT = nc.dram_tensor("xembT", [D, TPC], f32, kind="ExternalInput")
    wq = nc.dram_tensor("wq", [L, P, DC, D], bf16, kind="ExternalInput")
    wk = nc.dram_tensor("wk", [L, P, DC, D], bf16, kind="ExternalInput")
    wv = nc.dram_tensor("wv", [L, P, DC, D], bf16, kind="ExternalInput")
    wo = nc.dram_tensor("wo", [L, P, DC, D], bf16, kind="ExternalInput")
    w1 = nc.dram_tensor("w1", [L, FC, P, DC, P], bf16, kind="ExternalInput")
    w2 = nc.dram_tensor("w2", [L, DC, P, FC, P], bf16, kind="ExternalInput")
    wlm = nc.dram_tensor("wlm", [NG, P, DC, GV * 512], bf16, kind="ExternalInput")
    bq_d = nc.dram_tensor("bq", [L, P, DC], f32, kind="ExternalInput")
    bk_d = nc.dram_tensor("bk", [L, P, DC], f32, kind="ExternalInput")
    bo_d = nc.dram_tensor("bo", [L, P, DC], f32, kind="ExternalInput")
    b1_d = nc.dram_tensor("b1", [L, P, FC], f32, kind="ExternalInput")
    b2_d = nc.dram_tensor("b2", [L, P, DC], f32, kind="ExternalInput")
    blm_d = nc.dram_tensor("blm", [VPAD], f32, kind="ExternalInput")
    maskl_d = nc.dram_tensor("maskl", [P, P], bf16, kind="ExternalInput")
    pbias_d = nc.dram_tensor("pbias", [P, 1], f32, kind="ExternalInput")
    out_d = nc.dram_tensor("out", [TPC, V], bf16, kind="ExternalOutput")

    kvloc_k = nc.dram_tensor("kvloc_k", [K_SZ], bf16)
    kvred_k = nc.dram_tensor("kvred_k", [K_SZ], bf16)
    kvloc_v = nc.dram_tensor("kvloc_v", [V_SZ], bf16)
    kvred_v = nc.dram_tensor("kvred_v", [V_SZ], bf16)
    groups = [[0, 1], [2, 3], [4, 5], [6, 7]]

    with tile.TileContext(nc) as tc:
        with (
            tc.tile_pool(name="cpool", bufs=1) as cpool,
            tc.tile_pool(name="csm", bufs=2) as csm,
        ):
            # ---- persistent / common tiles
            x = cpool.tile([P, DC, TPC], f32, name="x")
            xf = cpool.tile([P, DC, TPC], bf16, name="xf")
            maskl = cpool.tile([P, P], bf16, name="maskl")
            pb = cpool.tile([P, 1], f32, name="pb")
            ones1 = cpool.tile([1, P], f32, name="ones1")
            ones128b = cpool.tile([P, 1], bf16, name="ones128b")
            eps_t = cpool.tile([1, 1], f32, name="eps_t")
            nc.vector.memset(ones1[:], 1.0)
            nc.vector.memset(ones128b[:], 1.0)
            nc.vector.memset(eps_t[:], EPS)
            nc.sync.dma_start(maskl[:], maskl_d[:])
            nc.sync.dma_start(pb[:], pbias_d[:])
            nc.sync.dma_start(x[:], xembT.rearrange("(c p) t -> p c t", p=P))

            lstack = ExitStack()
            lpool = lstack.enter_context(tc.tile_pool(name="lpool", bufs=1))
            wbig = lstack.enter_context(tc.tile_pool(name="wbig", bufs=3))
            w1p = lstack.enter_context(tc.tile_pool(name="w1p", bufs=3))
            w2p = lstack.enter_context(tc.tile_pool(name="w2p", bufs=2))
            sexpp = lstack.enter_context(tc.tile_pool(name="sexpp", bufs=4))
            sumc = lstack.enter_context(tc.tile_pool(name="sumc", bufs=2))
            vsmc = lstack.enter_context(tc.tile_pool(name="vsmc", bufs=1))
            xcp = lstack.enter_context(tc.tile_pool(name="xcp", bufs=2))
            rsmp = lstack.enter_context(tc.tile_pool(name="rsmp", bufs=1))
            small = lstack.enter_context(tc.tile_pool(name="small", bufs=2))
            small1 = lstack.enter_context(tc.tile_pool(name="small1", bufs=1))
            lnb = lstack.enter_context(tc.tile_pool(name="lnb", bufs=2))
            pscore = lstack.enter_context(tc.tile_pool(name="pscore", bufs=3, space="PSUM"))
            pav = lstack.enter_context(tc.tile_pool(name="pav", bufs=2, space="PSUM"))
            pmm = lstack.enter_context(tc.tile_pool(name="pmm", bufs=2, space="PSUM"))
            pstat = lstack.enter_context(tc.tile_pool(name="pstat", bufs=1, space="PSUM"))
            if True:
                h = lpool.tile([P, DC, TPC], bf16, name="h")
                qbf = lpool.tile([P, DC, TPC], bf16, name="qbf")
                kst = lpool.tile([P, DC, TPC], bf16, name="kst")
                kpeer = lpool.tile([P, DC, TPC], bf16, name="kpeer")
                vst = lpool.tile([P, 4, VW], bf16, name="vst")
                vpeer = lpool.tile([P, 4, VW], bf16, name="vpeer")
                obf = lpool.tile([P, DC, TPC], bf16, name="obf")
                xbf = lpool.tile([P, DC, TPC], bf16, name="xbf")
                r = lpool.tile([P, 16, TPC], bf16, name="r")
                statp = pstat.tile([P, TPC], f32, tag="st", name="statp")
                sxp = statp[0:1, :]
                sqp = statp[64:65, :]
                # ones columns of V_aug, set once (data writes never touch them)
                nc.vector.memset(vst[:], 1.0)

                def stats_chunk(m, first, last, eng):
                    eng.tensor_copy(xbf[:, m, :], x[:, m, :])
                    sqb = xcp.tile([P, TPC], bf16, tag="sq", name=f"sqb_{m}")
                    nc.vector.tensor_mul(sqb[:], xbf[:, m, :], xbf[:, m, :])
                    nc.tensor.matmul(sxp, ones128b[:], xbf[:, m, :], start=first, stop=last,
                                     skip_group_check=True)
                    nc.tensor.matmul(sqp, ones128b[:], sqb[:], start=first, stop=last,
                                     skip_group_check=True)

                def ln_finish(nm):
                    mu = small1.tile([1, TPC], f32, tag="mu", name=f"mu_{nm}")
                    ex2 = small1.tile([1, TPC], f32, tag="ex2", name=f"ex2_{nm}")
                    nc.vector.tensor_scalar_mul(mu[:], sxp, 1.0 / D)
                    nc.vector.tensor_scalar_mul(ex2[:], sqp, 1.0 / D)
                    var = small1.tile([1, TPC], f32, tag="var", name=f"var_{nm}")
                    nc.vector.tensor_mul(var[:], mu[:], mu[:])
                    nc.vector.tensor_sub(var[:], ex2[:], var[:])
                    nc.scalar.activation(var[:], var[:], AF.Sqrt, bias=eps_t[:], scale=1.0)
                    rstd = small1.tile([1, TPC], f32, tag="rstd", name=f"rstd_{nm}")
                    nc.vector.reciprocal(rstd[:], var[:])
                    msb2 = small1.tile([1, TPC], f32, tag="msb2", name=f"msb2_{nm}")
                    nc.vector.tensor_mul(msb2[:], mu[:], rstd[:])
                    bc1 = pmm.tile([P, TPC], f32, tag="mm", name=f"bc1_{nm}")
                    nc.tensor.matmul(bc1[:], ones1[:], rstd[:], start=True, stop=True)
                    rsb = rsmp.tile([P, TPC], bf16, tag="rsb", name=f"rsb_{nm}")
                    nc.scalar.copy(rsb[:], bc1[:])
                    bc2 = pmm.tile([P, TPC], f32, tag="mm", name=f"bc2_{nm}")
                    nc.tensor.matmul(bc2[:], ones1[:], msb2[:], start=True, stop=True)
                    msb = rsmp.tile([P, TPC], bf16, tag="msb", name=f"msb_{nm}")
                    nc.scalar.copy(msb[:], bc2[:])
                    return rsb, msb

                def ln_apply(out_bf, rsb, msb):
                    for hf in range(2):
                        sl = slice(hf * 4, hf * 4 + 4)
                        nc.vector.tensor_mul(out_bf[:, sl, :], xbf[:, sl, :],
                                             rsb[:, None, :].to_broadcast([P, 4, TPC]))
                        nc.vector.tensor_sub(out_bf[:, sl, :], out_bf[:, sl, :],
                                             msb[:, None, :].to_broadcast([P, 4, TPC]))

                # ---- initial LN1 (layer 0)
                for m in range(DC):
                    stats_chunk(m, m == 0, m == DC - 1,
                                nc.gpsimd if m % 2 else nc.vector)
                rsb0, msb0 = ln_finish("l0")
                ln_apply(h, rsb0, msb0)

                for li in range(L):
                    bqt = lnb.tile([P, DC, 1], f32, tag="bq", name=f"bqt_{li}")
                    nc.sync.dma_start(bqt[:], bq_d[li][:, :, None])
                    bkt = lnb.tile([P, DC, 1], f32, tag="bk", name=f"bkt_{li}")
                    nc.sync.dma_start(bkt[:], bk_d[li][:, :, None])

                    # ---------- K projection, stage, collective ----------
                    for hf in range(2):
                        wkh = wbig.tile([P, DC, 512], bf16, tag="w", name=f"wk_{li}_{hf}")
                        nc.scalar.dma_start(wkh[:], wk[li, :, :, hf * 512:(hf + 1) * 512])
                        for mm_ in range(4):
                            m = hf * 4 + mm_
                            ps = pmm.tile([P, TPC], f32, tag="mm", name=f"kps_{li}_{m}")
                            for c in range(DC):
                                nc.tensor.matmul(ps[:], wkh[:, c, mm_ * P:(mm_ + 1) * P],
                                                 h[:, c, :], start=(c == 0), stop=(c == DC - 1))
                            nc.scalar.activation(kst[:, m, :], ps[:], AF.Identity, bias=bkt[:, m])
                    nc.sync.dma_start(
                        kvloc_k.rearrange("(p c t) -> p c t", c=DC, t=TPC), kst[:])
                    nc.gpsimd.collective_compute(
                        "AllReduce", OP.add, replica_groups=groups,
                        ins=[kvloc_k[:]], outs=[kvred_k[:]])

                    # ---------- V projection, stage, collective ----------
                    for hf in range(2):
                        wvh = wbig.tile([P, DC, 512], bf16, tag="w", name=f"wv_{li}_{hf}")
                        nc.scalar.dma_start(wvh[:], wv[li, :, :, hf * 512:(hf + 1) * 512])
                        for tc4 in range(4):
                            ps = pmm.tile([P, TPC], f32, tag="mm", name=f"vps_{li}_{hf}_{tc4}")
                            for c in range(DC):
                                nc.tensor.matmul(
                                    ps[:], h[:, c, tc4 * P:(tc4 + 1) * P],
                                    wvh[:, c, :], start=(c == 0), stop=(c == DC - 1))
                            dst = vst[:, tc4, :].rearrange("p (h e) -> p h e", e=HS + 1)
                            if tc4 % 2:
                                nc.vector.tensor_copy(
                                    dst[:, hf * 8:(hf + 1) * 8, 0:HS],
                                    ps[:].rearrange("p (h e) -> p h e", e=HS))
                            else:
                                nc.scalar.copy(
                                    dst[:, hf * 8:(hf + 1) * 8, 0:HS],
                                    ps[:].rearrange("p (h e) -> p h e", e=HS))
                    nc.sync.dma_start(
                        kvloc_v.rearrange("(p c t) -> p c t", c=4, t=VW), vst[:])
                    nc.gpsimd.collective_compute(
                        "AllReduce", OP.add, replica_groups=groups,
                        ins=[kvloc_v[:]], outs=[kvred_v[:]])

                    # ---------- Q projection ----------
                    for hf in range(2):
                        wqh = wbig.tile([P, DC, 512], bf16, tag="w", name=f"wq_{li}_{hf}")
                        nc.scalar.dma_start(wqh[:], wq[li, :, :, hf * 512:(hf + 1) * 512])
                        for mm_ in range(4):
                            m = hf * 4 + mm_
                            ps = pmm.tile([P, TPC], f32, tag="mm", name=f"qps_{li}_{m}")
                            for c in range(DC):
                                nc.tensor.matmul(ps[:], wqh[:, c, mm_ * P:(mm_ + 1) * P],
                                                 h[:, c, :], start=(c == 0), stop=(c == DC - 1))
                            nc.scalar.activation(qbf[:, m, :], ps[:], AF.Identity, bias=bqt[:, m])

                    # ---------- peer K/V recovery (chunked) ----------
                    for c in range(DC):
                        ks = sumc.tile([P, TPC], bf16, tag="ks", name=f"ks_{li}_{c}")
                        nc.sync.dma_start(
                            ks[:], kvred_k.rearrange("(p c t) -> p c t", c=DC, t=TPC)[:, c, :])
                        nc.vector.tensor_sub(kpeer[:, c, :], ks[:], kst[:, c, :])
                    for tc4 in range(4):
                        vs = vsmc.tile([P, VW], bf16, tag="vs", name=f"vs_{li}_{tc4}")
                        nc.sync.dma_start(
                            vs[:], kvred_v.rearrange("(p c t) -> p c t", c=4, t=VW)[:, tc4, :])
                        nc.vector.tensor_sub(vpeer[:, tc4, :], vs[:], vst[:, tc4, :])

                    # ---------- attention ----------
                    def head_scores(hd):
                        hp = (hd % 2) * HS
                        hc = hd // 2
                        sexp = sexpp.tile([P, DC, TPC], bf16, tag="sx", name=f"sexp_{li}_{hd}")
                        for kt in range(4):
                            nq = TPC - kt * P
                            ps = pscore.tile([P, TPC], f32, tag="sc", name=f"sL_{li}_{hd}_{kt}")
                            nc.tensor.matmul(ps[:, 0:nq], kst[hp:hp + HS, hc, kt * P:(kt + 1) * P],
                                             qbf[hp:hp + HS, hc, kt * P:], start=True, stop=True)
                            nc.scalar.activation(sexp[:, kt, kt * P:], ps[:, 0:nq],
                                                 AF.Exp, scale=HS ** -0.5)
                            nc.vector.tensor_mul(sexp[:, kt, kt * P:(kt + 1) * P],
                                                 sexp[:, kt, kt * P:(kt + 1) * P], maskl[:])
                        for kt in range(4):
                            ps = pscore.tile([P, TPC], f32, tag="sc", name=f"sR_{li}_{hd}_{kt}")
                            nc.tensor.matmul(ps[:], kpeer[hp:hp + HS, hc, kt * P:(kt + 1) * P],
                                             qbf[hp:hp + HS, hc, :], start=True, stop=True)
                            nc.scalar.activation(sexp[:, 4 + kt, :], ps[:], AF.Exp,
                                                 scale=HS ** -0.5, bias=pb[:])
                        return sexp

                    def head_av(hd, sexp):
                        hp = (hd % 2) * HS
                        hc = hd // 2
                        av = pav.tile([P, TPC], f32, tag="av", name=f"av_{li}_{hd}")
                        for kt in range(4):
                            nc.tensor.matmul(av[0:HS + 1, kt * P:], vst[:, kt, hd * 65:hd * 65 + 65],
                                             sexp[:, kt, kt * P:], start=(kt == 0), stop=False,
                                             skip_group_check=True)
                        for kt in range(4):
                            nc.tensor.matmul(av[0:HS + 1, :], vpeer[:, kt, hd * 65:hd * 65 + 65],
                                             sexp[:, 4 + kt, :], start=False, stop=(kt == 3),
                                             skip_group_check=True)
                        rc = small.tile([1, TPC], f32, tag="rc", name=f"rc_{li}_{hd}")
                        nc.vector.reciprocal(rc[:], av[HS:HS + 1, :])
                        bc = pmm.tile([P, TPC], f32, tag="mm", name=f"bcp_{li}_{hd}")
                        nc.tensor.matmul(bc[0:HS, :], ones1[:, 0:HS], rc[:], start=True, stop=True)
                        bcs = small.tile([HS, TPC], f32, tag="bcs", name=f"bcs_{li}_{hd}")
                        nc.vector.tensor_copy(bcs[:], bc[0:HS, :])
                        nc.vector.tensor_mul(obf[hp:hp + HS, hc, :], av[0:HS, :], bcs[:])

                    prev = None
                    for hd in range(H + 1):
                        if hd < H:
                            se = head_scores(hd)
                        if prev is not None:
                            head_av(hd - 1, prev)
                        prev = se if hd < H else None

                    # ---------- O projection + residual + LN2 stats ----------
                    bot = lnb.tile([P, DC, 1], f32, tag="bo", name=f"bot_{li}")
                    nc.sync.dma_start(bot[:], bo_d[li][:, :, None])
                    for hf in range(2):
                        woh = wbig.tile([P, DC, 512], bf16, tag="w", name=f"wo_{li}_{hf}")
                        nc.scalar.dma_start(woh[:], wo[li, :, :, hf * 512:(hf + 1) * 512])
                        for mm_ in range(4):
                            m = hf * 4 + mm_
                            ps = pmm.tile([P, TPC], f32, tag="mm", name=f"ops_{li}_{m}")
                            for c in range(DC):
                                nc.tensor.matmul(ps[:], woh[:, c, mm_ * P:(mm_ + 1) * P],
                                                 obf[:, c, :], start=(c == 0), stop=(c == DC - 1))
                            nc.vector.scalar_tensor_tensor(
                                x[:, m, :], ps[:], bot[:, m], x[:, m, :], op0=OP.add, op1=OP.add)
                            stats_chunk(m, m == 0, m == DC - 1,
                                        nc.gpsimd if m % 2 else nc.vector)
                    rsb2, msb2s = ln_finish(f"l2_{li}")
                    ln_apply(h, rsb2, msb2s)

                    # ---------- MLP (two halves of DFF) ----------
                    b1t = lnb.tile([P, FC, 1], f32, tag="b1", name=f"b1t_{li}")
                    nc.sync.dma_start(b1t[:], b1_d[li][:, :, None])
                    b2t = lnb.tile([P, DC, 1], f32, tag="b2", name=f"b2t_{li}")
                    nc.sync.dma_start(b2t[:], b2_d[li][:, :, None])
                    for fh in range(2):
                        for mfl in range(16):
                            mf = fh * 16 + mfl
                            w1t = w1p.tile([P, DC, P], bf16, tag="w1", name=f"w1_{li}_{mf}")
                            nc.scalar.dma_start(w1t[:], w1[li, mf])
                            ps = pmm.tile([P, TPC], f32, tag="mm", name=f"mps_{li}_{mf}")
                            for c in range(DC):
                                nc.tensor.matmul(ps[:], w1t[:, c, :], h[:, c, :],
                                                 start=(c == 0), stop=(c == DC - 1))
                            nc.scalar.activation(r[:, mfl, :], ps[:], AF.Relu,
                                                 bias=b1t[:, mf], scale=1.0)
                        for m in range(DC):
                            w2t = w2p.tile([P, 16, P], bf16, tag="w2", name=f"w2_{li}_{fh}_{m}")
                            nc.scalar.dma_start(w2t[:], w2[li, m, :, fh * 16:(fh + 1) * 16, :])
                            ps = pmm.tile([P, TPC], f32, tag="mm", name=f"m2_{li}_{fh}_{m}")
                            for f in range(16):
                                nc.tensor.matmul(ps[:], w2t[:, f, :], r[:, f, :],
                                                 start=(f == 0), stop=(f == 15))
                            if fh == 0:
                                nc.vector.scalar_tensor_tensor(
                                    x[:, m, :], ps[:], b2t[:, m], x[:, m, :],
                                    op0=OP.add, op1=OP.add)
                            else:
                                nc.vector.tensor_add(x[:, m, :], x[:, m, :], ps[:])
                                stats_chunk(m, m == 0, m == DC - 1,
                                            nc.gpsimd if m % 2 else nc.vector)
                    if li < L - 1:
                        rsb1, msb1 = ln_finish(f"l1_{li + 1}")
                        ln_apply(h, rsb1, msb1)

                # ---------- final LN ----------
                rsbf, msbf = ln_finish("lf")
                ln_apply(xf, rsbf, msbf)

            # ---------- LM head ----------
            lstack.close()
            lmstack = ExitStack()
            wg = lmstack.enter_context(tc.tile_pool(name="wg", bufs=2))
            otp = lmstack.enter_context(tc.tile_pool(name="otp", bufs=4))
            blsp = lmstack.enter_context(tc.tile_pool(name="blsp", bufs=2))
            pacc = lmstack.enter_context(tc.tile_pool(name="pacc", bufs=4, space="PSUM"))
            pbc = lmstack.enter_context(tc.tile_pool(name="pbc", bufs=2, space="PSUM"))
            if True:
                for g in range(NG):
                    g0 = g * GV
                    gn = GV
                    wgt = wg.tile([P, DC, GV * 512], bf16, tag="wg", name=f"wg_{g}")
                    nc.scalar.dma_start(wgt[:], wlm[g])
                    blg = blsp.tile([1, GV * 512], f32, tag="blg", name=f"blg_{g}")
                    nc.sync.dma_start(blg[:], blm_d[None, g0 * 512:(g0 + gn) * 512])
                    blsts = []
                    for vi in range(gn):
                        bcp = pbc.tile([P, 512], f32, tag="bc", name=f"bcp_{g}_{vi}")
                        nc.tensor.matmul(bcp[:], ones1[:], blg[0:1, vi * 512:(vi + 1) * 512],
                                         start=True, stop=True)
                        blst = blsp.tile([P, 512], f32, tag=f"bls{vi}", name=f"bls_{g}_{vi}")
                        nc.scalar.activation(blst[:], bcp[:], AF.Copy)
                        blsts.append(blst)
                    for tc4 in range(4):
                        for vi in range(gn):
                            vc = g0 + vi
                            ps = pacc.tile([P, 512], f32, tag="acc", name=f"lm_{g}_{tc4}_{vi}")
                            for c in range(DC):
                                nc.tensor.matmul(
                                    ps[:], xf[:, c, tc4 * P:(tc4 + 1) * P],
                                    wgt[:, c, vi * 512:(vi + 1) * 512],
                                    start=(c == 0), stop=(c == DC - 1))
                            if vc >= NVC:
                                continue
                            nv = min(512, V - vc * 512)
                            ott = otp.tile([P, 512], bf16, tag="ot", name=f"ot_{g}_{tc4}_{vi}")
                            nc.vector.scalar_tensor_tensor(
                                ott[:], ps[:], 1.0, blsts[vi][:],
                                op0=OP.mult, op1=OP.add)
                            nc.sync.dma_start(
                                out_d[tc4 * P:(tc4 + 1) * P, vc * 512:vc * 512 + nv],
                                ott[:, 0:nv])
            lmstack.close()

    nc.compile()
    return nc


def kernel(**inputs):
    global LAST_EXEC_NS
    _install_ntff_hook()
    if "nc" not in _CACHE:
        _CACHE["nc"] = _build()
    nc = _CACHE["nc"]

    gi = {k: np.asarray(v, np.float32) if np.asarray(v).dtype == np.float32
          else np.asarray(v) for k, v in inputs.items()}
    idx = np.asarray(gi["idx"]).astype(np.int64)
    xemb = np.asarray(gi["wte"])[idx] + np.asarray(gi["wpe"])[:T][None, :, :]

    # ---- fold LN weights/biases into adjacent projections (host, fp32)
    ln1w = np.asarray(gi["ln1_w"]); ln1b = np.asarray(gi["ln1_b"])
    ln2w = np.asarray(gi["ln2_w"]); ln2b = np.asarray(gi["ln2_b"])
    lnfw = np.asarray(gi["lnf_w"]); lnfb = np.asarray(gi["lnf_b"])
    wq_e = ln1w[:, :, None] * gi["wq"]          # [L,D,D]
    wk_e = ln1w[:, :, None] * gi["wk"]
    wv_e = ln1w[:, :, None] * gi["wv"]
    bq_v = np.einsum('ld,lde->le', ln1b, gi["wq"])   # [L,D]
    bk_v = np.einsum('ld,lde->le', ln1b, gi["wk"])
    bv_v = np.einsum('ld,lde->le', ln1b, gi["wv"])
    bo_e = gi["bo"] + np.einsum('ld,lde->le', bv_v, gi["wo"])
    w1_e = ln2w[:, :, None] * gi["w1"]
    b1_e = gi["b1"] + np.einsum('ld,lde->le', ln2b, gi["w1"])
    wlm_e = lnfw[:, None] * gi["wlm"]
    blm_e = gi["blm"] + lnfb @ gi["wlm"]

    def pack_sq(w):   # [L, 1024, N] -> [L, 128, 8, N]
        Lw, Kw, Nw = w.shape
        return np.ascontiguousarray(
            w.reshape(Lw, DC, P, Nw).transpose(0, 2, 1, 3).astype(ml_dtypes.bfloat16))

    w1p = np.ascontiguousarray(
        w1_e.reshape(L, DC, P, FC, P).transpose(0, 3, 2, 1, 4).astype(ml_dtypes.bfloat16))
    w2p = np.ascontiguousarray(
        np.asarray(gi["w2"]).reshape(L, FC, P, DC, P).transpose(0, 3, 2, 1, 4)
        .astype(ml_dtypes.bfloat16))
    wlmp = np.zeros((D, VPAD), np.float32)
    wlmp[:, :V] = wlm_e
    wlmp = np.ascontiguousarray(
        wlmp.reshape(DC, P, NG, GV * 512).transpose(2, 1, 0, 3).astype(ml_dtypes.bfloat16))
    blmp = np.zeros((VPAD,), np.float32)
    blmp[:V] = blm_e

    def packv(v):  # [.., N] -> [.., P, N//P]
        v = np.asarray(v, np.float32)
        nch = v.shape[-1] // P
        return np.ascontiguousarray(
            v.reshape(v.shape[:-1] + (nch, P)).swapaxes(-1, -2))

    # diagonal-block causal triangle: same on every core and every chunk
    ml_m = (np.arange(P)[:, None] <= np.arange(P)[None, :]).astype(np.float32)

    shared = dict(
        wq=pack_sq(wq_e), wk=pack_sq(wk_e), wv=pack_sq(wv_e),
        wo=pack_sq(np.asarray(gi["wo"], np.float32)),
        w1=w1p, w2=w2p, wlm=wlmp,
        bq=packv(bq_v), bk=packv(bk_v), bo=packv(bo_e),
        b1=packv(b1_e), b2=packv(np.asarray(gi["b2"], np.float32)),
        blm=np.ascontiguousarray(blmp),
        maskl=ml_m.astype(ml_dtypes.bfloat16),
    )

    in_maps = []
    for c in range(8):
        b, half = c // 2, c % 2
        sl = slice(half * TPC, (half + 1) * TPC)
        im = dict(shared)
        im["xembT"] = np.ascontiguousarray(xemb[b, sl].T, dtype=np.float32)
        im["pbias"] = np.full((P, 1), 0.0 if half else -60000.0, np.float32)
        in_maps.append(im)

    res = run_bass_kernel_spmd(nc, in_maps, list(range(8)),
                               trace=bool(os.environ.get("BASS_TRACE")))
    LAST_EXEC_NS = res.exec_time_ns

    out = np.empty((B, T, V), np.float32)
    for c in range(8):
        b, half = c // 2, c % 2
        out[b, half * TPC:(half + 1) * TPC] = res.results[c]["out"].astype(np.float32)
    return out


# revision 14
# speedup vs baseline: 1.3497x; 1.0736x over previous
"""GPT-2 (L=8, D=1024, H=16, V=50257, B=4, T=1024) forward on 8 TRN2 NeuronCores.

Sharding: core c handles batch b=c//2, sequence half h=c%2 (512 tokens).
Weights replicated (bf16). Per layer, K/V are exchanged between the two cores
of a batch-pair with an AllReduce(add); each core recovers the peer half by
subtracting its own contribution (bf16 sub). Attention chunk order is
core-relative: chunks 0-3 = local keys (direct from SBUF, no collective wait),
chunks 4-7 = peer keys. Causality is data-driven: a diagonal [128,4,512] mask
(identical on all cores) for the local half, and a per-core exp bias
(0 or -60000) that zeroes the whole peer half on first-half cores.

LN weights/biases are folded into the adjacent projection weights host-side,
so on-chip LN is a pure (x-mu)*rstd; stats are accumulated chunk-by-chunk as
the residual stream is produced. LM head runs in vocab groups of 6 sharing
the stationary activations across 6 PSUM banks, bf16 output (host upcasts).
"""

import os
import sys
import types
from contextlib import ExitStack

import numpy as np
import ml_dtypes

import concourse.bass as bass
import concourse.mybir as mybir
import concourse.tile as tile
from concourse import bacc
from concourse.bass_utils import run_bass_kernel_spmd

f32 = mybir.dt.float32
bf16 = mybir.dt.bfloat16
AF = mybir.ActivationFunctionType
OP = mybir.AluOpType

L, D, H, V, DFF = 8, 1024, 16, 50257, 4096
HS = D // H          # 64
B, T = 4, 1024
TPC = 512            # tokens per core
P = 128
DC = D // P          # 8 d-chunks
FC = DFF // P        # 32 dff-chunks
NVC = (V + 511) // 512   # 99 vocab chunks
GV = 6               # lm-head vocab chunks per group
NG = (NVC + GV - 1) // GV        # 17 groups
NVC2 = NG * GV                   # 102 (padded)
VPAD = NVC2 * 512
EPS = 1e-5
VW = H * (HS + 1)    # 1040

K_SZ = P * DC * TPC           # 524288
V_SZ = P * 4 * VW             # 532480

LAST_EXEC_NS = None
_CACHE = {}


def _install_ntff_hook():
    """Provide antenv.axon_hooks if the image lacks it, so trace=True works."""
    try:
        import antenv
        try:
            from antenv import axon_hooks  # noqa: F401
            return
        except ImportError:
            pass
        hooks_mod = types.ModuleType("antenv.axon_hooks")
        _hook = [None]
        hooks_mod.set_axon_ntff_profile_hook = lambda h: _hook.__setitem__(0, h)
        hooks_mod.get_axon_ntff_profile_hook = lambda: _hook[0]
        sys.modules["antenv.axon_hooks"] = hooks_mod
        antenv.axon_hooks = hooks_mod
        from trn_agent_boot.trn_boot import _ntff_profile_via_ctypes
        hooks_mod.set_axon_ntff_profile_hook(
            _ntff_profile_via_ctypes("/opt/axon/libaxon_pjrt.so"))
    except Exception:
        pass


def _build():
    nc = bacc.Bacc(None, target_bir_lowering=False, debug=False)

    xembT = nc.dram_tensor("xembT", [D, TPC], f32, kind="ExternalInput")
    wq = nc.dram_tensor("wq", [L, P, DC, D], bf16, kind="ExternalInput")
    wk = nc.dram_tensor("wk", [L, P, DC, D], bf16, kind="ExternalInput")
    wv = nc.dram_tensor("wv", [L, P, DC, D], bf16, kind="ExternalInput")
    wo = nc.dram_tensor("wo", [L, P, DC, D], bf16, kind="ExternalInput")
    w1 = nc.dram_tensor("w1", [L, FC, P, DC, P], bf16, kind="ExternalInput")
    w2 = nc.dram_tensor("w2", [L, DC, P, FC, P], bf16, kind="ExternalInput")
    wlm = nc.dram_tensor("wlm", [NG, P, DC, GV * 512], bf16, kind="ExternalInput")
    bq_d = nc.dram_tensor("bq", [L, P, DC], f32, kind="ExternalInput")
    bk_d = nc.dram_tensor("bk", [L, P, DC], f32, kind="ExternalInput")
    bo_d = nc.dram_tensor("bo", [L, P, DC], f32, kind="ExternalInput")
    b1_d = nc.dram_tensor("b1", [L, P, FC], f32, kind="ExternalInput")
    b2_d = nc.dram_tensor("b2", [L, P, DC], f32, kind="ExternalInput")
    blm_d = nc.dram_tensor("blm", [VPAD], f32, kind="ExternalInput")
    maskl_d = nc.dram_tensor("maskl", [P, P], bf16, kind="ExternalInput")
    pbias_d = nc.dram_tensor("pbias", [P, 1], f32, kind="ExternalInput")
    out_d = nc.dram_tensor("out", [TPC, V], bf16, kind="ExternalOutput")

    kvloc_k = nc.dram_tensor("kvloc_k", [K_SZ], bf16)
    kvred_k = nc.dram_tensor("kvred_k", [K_SZ], bf16)
    kvloc_v = nc.dram_tensor("kvloc_v", [V_SZ], bf16)
    kvred_v = nc.dram_tensor("kvred_v", [V_SZ], bf16)
    groups = [[0, 1], [2, 3], [4, 5], [6, 7]]

    with tile.TileContext(nc) as tc:
        with (
            tc.tile_pool(name="cpool", bufs=1) as cpool,
            tc.tile_pool(name="csm", bufs=2) as csm,
        ):
            # ---- persistent / common tiles
            x = cpool.tile([P, DC, TPC], f32, name="x")
            xf = cpool.tile([P, DC, TPC], bf16, name="xf")
            maskl = cpool.tile([P, P], bf16, name="maskl")
            pb = cpool.tile([P, 1], f32, name="pb")
            ones1 = cpool.tile([1, P], f32, name="ones1")
            ones128b = cpool.tile([P, 1], bf16, name="ones128b")
            eps_t = cpool.tile([1, 1], f32, name="eps_t")
            nc.vector.memset(ones1[:], 1.0)
            nc.vector.memset(ones128b[:], 1.0)
            nc.vector.memset(eps_t[:], EPS)
            nc.sync.dma_start(maskl[:], maskl_d[:])
            nc.sync.dma_start(pb[:], pbias_d[:])
            nc.sync.dma_start(x[:], xembT.rearrange("(c p) t -> p c t", p=P))

            lstack = ExitStack()
            lpool = lstack.enter_context(tc.tile_pool(name="lpool", bufs=1))
            wbig = lstack.enter_context(tc.tile_pool(name="wbig", bufs=3))
            w1p = lstack.enter_context(tc.tile_pool(name="w1p", bufs=3))
            w2p = lstack.enter_context(tc.tile_pool(name="w2p", bufs=2))
            sexpp = lstack.enter_context(tc.tile_pool(name="sexpp", bufs=4))
            sumc = lstack.enter_context(tc.tile_pool(name="sumc", bufs=2))
            vsmc = lstack.enter_context(tc.tile_pool(name="vsmc", bufs=1))
            xcp = lstack.enter_context(tc.tile_pool(name="xcp", bufs=2))
            rsmp = lstack.enter_context(tc.tile_pool(name="rsmp", bufs=1))
            small = lstack.enter_context(tc.tile_pool(name="small", bufs=2))
            small1 = lstack.enter_context(tc.tile_pool(name="small1", bufs=1))
            lnb = lstack.enter_context(tc.tile_pool(name="lnb", bufs=2))
            pscore = lstack.enter_context(tc.tile_pool(name="pscore", bufs=3, space="PSUM"))
            pav = lstack.enter_context(tc.tile_pool(name="pav", bufs=2, space="PSUM"))
            pmm = lstack.enter_context(tc.tile_pool(name="pmm", bufs=2, space="PSUM"))
            pstat = lstack.enter_context(tc.tile_pool(name="pstat", bufs=1, space="PSUM"))
            if True:
                h = lpool.tile([P, DC, TPC], bf16, name="h")
                qbf = lpool.tile([P, DC, TPC], bf16, name="qbf")
                kst = lpool.tile([P, DC, TPC], bf16, name="kst")
                kpeer = lpool.tile([P, DC, TPC], bf16, name="kpeer")
                vst = lpool.tile([P, 4, VW], bf16, name="vst")
                vpeer = lpool.tile([P, 4, VW], bf16, name="vpeer")
                obf = lpool.tile([P, DC, TPC], bf16, name="obf")
                xbf = lpool.tile([P, DC, TPC], bf16, name="xbf")
                r = lpool.tile([P, 16, TPC], bf16, name="r")
                statp = pstat.tile([P, TPC], f32, tag="st", name="statp")
                sxp = statp[0:1, :]
                sqp = statp[64:65, :]
                # ones columns of V_aug, set once (data writes never touch them)
                nc.vector.memset(vst[:], 1.0)

                def stats_chunk(m, first, last, eng):
                    eng.tensor_copy(xbf[:, m, :], x[:, m, :])
                    sqb = xcp.tile([P, TPC], bf16, tag="sq", name=f"sqb_{m}")
                    nc.vector.tensor_mul(sqb[:], xbf[:, m, :], xbf[:, m, :])
                    nc.tensor.matmul(sxp, ones128b[:], xbf[:, m, :], start=first, stop=last,
                                     skip_group_check=True)
                    nc.tensor.matmul(sqp, ones128b[:], sqb[:], start=first, stop=last,
                                     skip_group_check=True)

                def ln_finish(nm):
                    mu = small1.tile([1, TPC], f32, tag="mu", name=f"mu_{nm}")
                    ex2 = small1.tile([1, TPC], f32, tag="ex2", name=f"ex2_{nm}")
                    nc.vector.tensor_scalar_mul(mu[:], sxp, 1.0 / D)
                    nc.vector.tensor_scalar_mul(ex2[:], sqp, 1.0 / D)
                    var = small1.tile([1, TPC], f32, tag="var", name=f"var_{nm}")
                    nc.vector.tensor_mul(var[:], mu[:], mu[:])
                    nc.vector.tensor_sub(var[:], ex2[:], var[:])
                    nc.scalar.activation(var[:], var[:], AF.Sqrt, bias=eps_t[:], scale=1.0)
                    rstd = small1.tile([1, TPC], f32, tag="rstd", name=f"rstd_{nm}")
                    nc.vector.reciprocal(rstd[:], var[:])
                    msb2 = small1.tile([1, TPC], f32, tag="msb2", name=f"msb2_{nm}")
                    nc.vector.tensor_mul(msb2[:], mu[:], rstd[:])
                    bc1 = pmm.tile([P, TPC], f32, tag="mm", name=f"bc1_{nm}")
                    nc.tensor.matmul(bc1[:], ones1[:], rstd[:], start=True, stop=True)
                    rsb = rsmp.tile([P, TPC], bf16, tag="rsb", name=f"rsb_{nm}")
                    nc.scalar.copy(rsb[:], bc1[:])
                    bc2 = pmm.tile([P, TPC], f32, tag="mm", name=f"bc2_{nm}")
                    nc.tensor.matmul(bc2[:], ones1[:], msb2[:], start=True, stop=True)
                    msb = rsmp.tile([P, TPC], bf16, tag="msb", name=f"msb_{nm}")
                    nc.scalar.copy(msb[:], bc2[:])
                    return rsb, msb

                def ln_apply(out_bf, rsb, msb):
                    for hf in range(2):
                        sl = slice(hf * 4, hf * 4 + 4)
                        nc.vector.tensor_mul(out_bf[:, sl, :], xbf[:, sl, :],
                                             rsb[:, None, :].to_broadcast([P, 4, TPC]))
                        nc.vector.tensor_sub(out_bf[:, sl, :], out_bf[:, sl, :],
                                             msb[:, None, :].to_broadcast([P, 4, TPC]))

                # ---- initial LN1 (layer 0)
                for m in range(DC):
                    stats_chunk(m, m == 0, m == DC - 1,
                                nc.gpsimd if m % 2 else nc.vector)
                rsb0, msb0 = ln_finish("l0")
                ln_apply(h, rsb0, msb0)

                for li in range(L):
                    bqt = lnb.tile([P, DC, 1], f32, tag="bq", name=f"bqt_{li}")
                    nc.sync.dma_start(bqt[:], bq_d[li][:, :, None])
                    bkt = lnb.tile([P, DC, 1], f32, tag="bk", name=f"bkt_{li}")
                    nc.sync.dma_start(bkt[:], bk_d[li][:, :, None])

                    # ---------- weight prefetch (scalar DGE queue) ----------
                    wkhs, wqhs, wvhs = [], [], []
                    for hf in range(2):
                        wkh = wbig.tile([P, DC, 512], bf16, tag="w", name=f"wk_{li}_{hf}")
                        nc.scalar.dma_start(wkh[:], wk[li, :, :, hf * 512:(hf + 1) * 512])
                        wkhs.append(wkh)
                    for hf in range(2):
                        wqh = wbig.tile([P, DC, 512], bf16, tag="w", name=f"wq_{li}_{hf}")
                        nc.scalar.dma_start(wqh[:], wq[li, :, :, hf * 512:(hf + 1) * 512])
                        wqhs.append(wqh)
                    for hf in range(2):
                        wvh = wbig.tile([P, DC, 512], bf16, tag="w", name=f"wv_{li}_{hf}")
                        nc.scalar.dma_start(wvh[:], wv[li, :, :, hf * 512:(hf + 1) * 512])
                        wvhs.append(wvh)

                    # ---------- K projection, stage, collective ----------
                    for hf in range(2):
                        for mm_ in range(4):
                            m = hf * 4 + mm_
                            ps = pmm.tile([P, TPC], f32, tag="mm", name=f"kps_{li}_{m}")
                            for c in range(DC):
                                nc.tensor.matmul(ps[:], wkhs[hf][:, c, mm_ * P:(mm_ + 1) * P],
                                                 h[:, c, :], start=(c == 0), stop=(c == DC - 1))
                            nc.scalar.activation(kst[:, m, :], ps[:], AF.Identity, bias=bkt[:, m])
                    nc.sync.dma_start(
                        kvloc_k.rearrange("(p c t) -> p c t", c=DC, t=TPC), kst[:])
                    nc.gpsimd.collective_compute(
                        "AllReduce", OP.add, replica_groups=groups,
                        ins=[kvloc_k[:]], outs=[kvred_k[:]])

                    # ---------- Q projection ----------
                    for hf in range(2):
                        for mm_ in range(4):
                            m = hf * 4 + mm_
                            ps = pmm.tile([P, TPC], f32, tag="mm", name=f"qps_{li}_{m}")
                            for c in range(DC):
                                nc.tensor.matmul(ps[:], wqhs[hf][:, c, mm_ * P:(mm_ + 1) * P],
                                                 h[:, c, :], start=(c == 0), stop=(c == DC - 1))
                            nc.scalar.activation(qbf[:, m, :], ps[:], AF.Identity, bias=bqt[:, m])

                    # ---------- V projection, stage, collective ----------
                    for hf in range(2):
                        for tc4 in range(4):
                            ps = pmm.tile([P, TPC], f32, tag="mm", name=f"vps_{li}_{hf}_{tc4}")
                            for c in range(DC):
                                nc.tensor.matmul(
                                    ps[:], h[:, c, tc4 * P:(tc4 + 1) * P],
                                    wvhs[hf][:, c, :], start=(c == 0), stop=(c == DC - 1))
                            dst = vst[:, tc4, :].rearrange("p (h e) -> p h e", e=HS + 1)
                            if tc4 % 2:
                                nc.vector.tensor_copy(
                                    dst[:, hf * 8:(hf + 1) * 8, 0:HS],
                                    ps[:].rearrange("p (h e) -> p h e", e=HS))
                            else:
                                nc.scalar.copy(
                                    dst[:, hf * 8:(hf + 1) * 8, 0:HS],
                                    ps[:].rearrange("p (h e) -> p h e", e=HS))
                    nc.sync.dma_start(
                        kvloc_v.rearrange("(p c t) -> p c t", c=4, t=VW), vst[:])
                    nc.gpsimd.collective_compute(
                        "AllReduce", OP.add, replica_groups=groups,
                        ins=[kvloc_v[:]], outs=[kvred_v[:]])

                    # ---------- peer K/V recovery (chunked) ----------
                    for c in range(DC):
                        ks = sumc.tile([P, TPC], bf16, tag="ks", name=f"ks_{li}_{c}")
                        nc.sync.dma_start(
                            ks[:], kvred_k.rearrange("(p c t) -> p c t", c=DC, t=TPC)[:, c, :])
                        nc.vector.tensor_sub(kpeer[:, c, :], ks[:], kst[:, c, :])
                    for tc4 in range(4):
                        vs = vsmc.tile([P, VW], bf16, tag="vs", name=f"vs_{li}_{tc4}")
                        nc.sync.dma_start(
                            vs[:], kvred_v.rearrange("(p c t) -> p c t", c=4, t=VW)[:, tc4, :])
                        nc.vector.tensor_sub(vpeer[:, tc4, :], vs[:], vst[:, tc4, :])

                    # ---------- attention ----------
                    def head_scores(hd):
                        hp = (hd % 2) * HS
                        hc = hd // 2
                        sexp = sexpp.tile([P, DC, TPC], bf16, tag="sx", name=f"sexp_{li}_{hd}")
                        for kt in range(4):
                            nq = TPC - kt * P
                            ps = pscore.tile([P, TPC], f32, tag="sc", name=f"sL_{li}_{hd}_{kt}")
                            nc.tensor.matmul(ps[:, 0:nq], kst[hp:hp + HS, hc, kt * P:(kt + 1) * P],
                                             qbf[hp:hp + HS, hc, kt * P:], start=True, stop=True)
                            nc.scalar.activation(sexp[:, kt, kt * P:], ps[:, 0:nq],
                                                 AF.Exp, scale=HS ** -0.5)
                            nc.vector.tensor_mul(sexp[:, kt, kt * P:(kt + 1) * P],
                                                 sexp[:, kt, kt * P:(kt + 1) * P], maskl[:])
                        for kt in range(4):
                            ps = pscore.tile([P, TPC], f32, tag="sc", name=f"sR_{li}_{hd}_{kt}")
                            nc.tensor.matmul(ps[:], kpeer[hp:hp + HS, hc, kt * P:(kt + 1) * P],
                                             qbf[hp:hp + HS, hc, :], start=True, stop=True)
                            nc.scalar.activation(sexp[:, 4 + kt, :], ps[:], AF.Exp,
                                                 scale=HS ** -0.5, bias=pb[:])
                        return sexp

                    def head_av(hd, sexp):
                        hp = (hd % 2) * HS
                        hc = hd // 2
                        av = pav.tile([P, TPC], f32, tag="av", name=f"av_{li}_{hd}")
                        for kt in range(4):
                            nc.tensor.matmul(av[0:HS + 1, kt * P:], vst[:, kt, hd * 65:hd * 65 + 65],
                                             sexp[:, kt, kt * P:], start=(kt == 0), stop=False,
                                             skip_group_check=True)
                        for kt in range(4):
                            nc.tensor.matmul(av[0:HS + 1, :], vpeer[:, kt, hd * 65:hd * 65 + 65],
                                             sexp[:, 4 + kt, :], start=False, stop=(kt == 3),
                                             skip_group_check=True)
                        rc = small.tile([1, TPC], f32, tag="rc", name=f"rc_{li}_{hd}")
                        nc.vector.reciprocal(rc[:], av[HS:HS + 1, :])
                        return av, rc

                    def head_norm(hd, av, rc):
                        hp = (hd % 2) * HS
                        hc = hd // 2
                        bc = pmm.tile([P, TPC], f32, tag="mm", name=f"bcp_{li}_{hd}")
                        nc.tensor.matmul(bc[0:HS, :], ones1[:, 0:HS], rc[:], start=True, stop=True)
                        bcs = small.tile([HS, TPC], f32, tag="bcs", name=f"bcs_{li}_{hd}")
                        nc.vector.tensor_copy(bcs[:], bc[0:HS, :])
                        nc.vector.tensor_mul(obf[hp:hp + HS, hc, :], av[0:HS, :], bcs[:])

                    se_p = av_p = None
                    for hd in range(H + 2):
                        se = head_scores(hd) if hd < H else None
                        if se_p is not None:
                            av_n = (hd - 1,) + head_av(hd - 1, se_p)
                        else:
                            av_n = None
                        if av_p is not None:
                            head_norm(*av_p)
                        se_p, av_p = se, av_n

                    # ---------- O projection + residual + LN2 stats ----------
                    bot = lnb.tile([P, DC, 1], f32, tag="bo", name=f"bot_{li}")
                    nc.sync.dma_start(bot[:], bo_d[li][:, :, None])
                    for hf in range(2):
                        woh = wbig.tile([P, DC, 512], bf16, tag="w", name=f"wo_{li}_{hf}")
                        nc.scalar.dma_start(woh[:], wo[li, :, :, hf * 512:(hf + 1) * 512])
                        for mm_ in range(4):
                            m = hf * 4 + mm_
                            ps = pmm.tile([P, TPC], f32, tag="mm", name=f"ops_{li}_{m}")
                            for c in range(DC):
                                nc.tensor.matmul(ps[:], woh[:, c, mm_ * P:(mm_ + 1) * P],
                                                 obf[:, c, :], start=(c == 0), stop=(c == DC - 1))
                            nc.vector.scalar_tensor_tensor(
                                x[:, m, :], ps[:], bot[:, m], x[:, m, :], op0=OP.add, op1=OP.add)
                            stats_chunk(m, m == 0, m == DC - 1,
                                        nc.gpsimd if m % 2 else nc.vector)
                    rsb2, msb2s = ln_finish(f"l2_{li}")
                    ln_apply(h, rsb2, msb2s)

                    # ---------- MLP (two halves of DFF) ----------
                    b1t = lnb.tile([P, FC, 1], f32, tag="b1", name=f"b1t_{li}")
                    nc.sync.dma_start(b1t[:], b1_d[li][:, :, None])
                    b2t = lnb.tile([P, DC, 1], f32, tag="b2", name=f"b2t_{li}")
                    nc.sync.dma_start(b2t[:], b2_d[li][:, :, None])
                    for fh in range(2):
                        for mfl in range(16):
                            mf = fh * 16 + mfl
                            w1t = w1p.tile([P, DC, P], bf16, tag="w1", name=f"w1_{li}_{mf}")
                            nc.scalar.dma_start(w1t[:], w1[li, mf])
                            ps = pmm.tile([P, TPC], f32, tag="mm", name=f"mps_{li}_{mf}")
                            for c in range(DC):
                                nc.tensor.matmul(ps[:], w1t[:, c, :], h[:, c, :],
                                                 start=(c == 0), stop=(c == DC - 1))
                            nc.scalar.activation(r[:, mfl, :], ps[:], AF.Relu,
                                                 bias=b1t[:, mf], scale=1.0)
                        for m in range(DC):
                            w2t = w2p.tile([P, 16, P], bf16, tag="w2", name=f"w2_{li}_{fh}_{m}")
                            nc.scalar.dma_start(w2t[:], w2[li, m, :, fh * 16:(fh + 1) * 16, :])
                            ps = pmm.tile([P, TPC], f32, tag="mm", name=f"m2_{li}_{fh}_{m}")
                            for f in range(16):
                                nc.tensor.matmul(ps[:], w2t[:, f, :], r[:, f, :],
                                                 start=(f == 0), stop=(f == 15))
                            if fh == 0:
                                nc.vector.scalar_tensor_tensor(
                                    x[:, m, :], ps[:], b2t[:, m], x[:, m, :],
                                    op0=OP.add, op1=OP.add)
                            else:
                                nc.vector.tensor_add(x[:, m, :], x[:, m, :], ps[:])
                                stats_chunk(m, m == 0, m == DC - 1,
                                            nc.gpsimd if m % 2 else nc.vector)
                    if li < L - 1:
                        rsb1, msb1 = ln_finish(f"l1_{li + 1}")
                        ln_apply(h, rsb1, msb1)

                # ---------- final LN ----------
                rsbf, msbf = ln_finish("lf")
                ln_apply(xf, rsbf, msbf)

            # ---------- LM head ----------
            lstack.close()
            lmstack = ExitStack()
            wg = lmstack.enter_context(tc.tile_pool(name="wg", bufs=2))
            otp = lmstack.enter_context(tc.tile_pool(name="otp", bufs=4))
            blsp = lmstack.enter_context(tc.tile_pool(name="blsp", bufs=2))
            pacc = lmstack.enter_context(tc.tile_pool(name="pacc", bufs=4, space="PSUM"))
            pbc = lmstack.enter_context(tc.tile_pool(name="pbc", bufs=2, space="PSUM"))
            if True:
                for g in range(NG):
                    g0 = g * GV
                    gn = GV
                    wgt = wg.tile([P, DC, GV * 512], bf16, tag="wg", name=f"wg_{g}")
                    nc.scalar.dma_start(wgt[:], wlm[g])
                    blg = blsp.tile([1, GV * 512], f32, tag="blg", name=f"blg_{g}")
                    nc.sync.dma_start(blg[:], blm_d[None, g0 * 512:(g0 + gn) * 512])
                    blsts = []
                    for vi in range(gn):
                        bcp = pbc.tile([P, 512], f32, tag="bc", name=f"bcp_{g}_{vi}")
                        nc.tensor.matmul(bcp[:], ones1[:], blg[0:1, vi * 512:(vi + 1) * 512],
                                         start=True, stop=True)
                        blst = blsp.tile([P, 512], f32, tag=f"bls{vi}", name=f"bls_{g}_{vi}")
                        nc.scalar.activation(blst[:], bcp[:], AF.Copy)
                        blsts.append(blst)
                    for tc4 in range(4):
                        for vi in range(gn):
                            vc = g0 + vi
                            ps = pacc.tile([P, 512], f32, tag="acc", name=f"lm_{g}_{tc4}_{vi}")
                            for c in range(DC):
                                nc.tensor.matmul(
                                    ps[:], xf[:, c, tc4 * P:(tc4 + 1) * P],
                                    wgt[:, c, vi * 512:(vi + 1) * 512],
                                    start=(c == 0), stop=(c == DC - 1))
                            if vc >= NVC:
                                continue
                            nv = min(512, V - vc * 512)
                            ott = otp.tile([P, 512], bf16, tag="ot", name=f"ot_{g}_{tc4}_{vi}")
                            nc.vector.scalar_tensor_tensor(
                                ott[:], ps[:], 1.0, blsts[vi][:],
                                op0=OP.mult, op1=OP.add)
                            nc.sync.dma_start(
                                out_d[tc4 * P:(tc4 + 1) * P, vc * 512:vc * 512 + nv],
                                ott[:, 0:nv])
            lmstack.close()

    nc.compile()
    return nc


def kernel(**inputs):
    global LAST_EXEC_NS
    _install_ntff_hook()
    if "nc" not in _CACHE:
        _CACHE["nc"] = _build()
    nc = _CACHE["nc"]

    gi = {k: np.asarray(v, np.float32) if np.asarray(v).dtype == np.float32
          else np.asarray(v) for k, v in inputs.items()}
    idx = np.asarray(gi["idx"]).astype(np.int64)
    xemb = np.asarray(gi["wte"])[idx] + np.asarray(gi["wpe"])[:T][None, :, :]

    # ---- fold LN weights/biases into adjacent projections (host, fp32)
    ln1w = np.asarray(gi["ln1_w"]); ln1b = np.asarray(gi["ln1_b"])
    ln2w = np.asarray(gi["ln2_w"]); ln2b = np.asarray(gi["ln2_b"])
    lnfw = np.asarray(gi["lnf_w"]); lnfb = np.asarray(gi["lnf_b"])
    wq_e = ln1w[:, :, None] * gi["wq"]          # [L,D,D]
    wk_e = ln1w[:, :, None] * gi["wk"]
    wv_e = ln1w[:, :, None] * gi["wv"]
    bq_v = np.einsum('ld,lde->le', ln1b, gi["wq"])   # [L,D]
    bk_v = np.einsum('ld,lde->le', ln1b, gi["wk"])
    bv_v = np.einsum('ld,lde->le', ln1b, gi["wv"])
    bo_e = gi["bo"] + np.einsum('ld,lde->le', bv_v, gi["wo"])
    w1_e = ln2w[:, :, None] * gi["w1"]
    b1_e = gi["b1"] + np.einsum('ld,lde->le', ln2b, gi["w1"])
    wlm_e = lnfw[:, None] * gi["wlm"]
    blm_e = gi["blm"] + lnfb @ gi["wlm"]

    def pack_sq(w):   # [L, 1024, N] -> [L, 128, 8, N]
        Lw, Kw, Nw = w.shape
        return np.ascontiguousarray(
            w.reshape(Lw, DC, P, Nw).transpose(0, 2, 1, 3).astype(ml_dtypes.bfloat16))

    w1p = np.ascontiguousarray(
        w1_e.reshape(L, DC, P, FC, P).transpose(0, 3, 2, 1, 4).astype(ml_dtypes.bfloat16))
    w2p = np.ascontiguousarray(
        np.asarray(gi["w2"]).reshape(L, FC, P, DC, P).transpose(0, 3, 2, 1, 4)
        .astype(ml_dtypes.bfloat16))
    wlmp = np.zeros((D, VPAD), np.float32)
    wlmp[:, :V] = wlm_e
    wlmp = np.ascontiguousarray(
        wlmp.reshape(DC, P, NG, GV * 512).transpose(2, 1, 0, 3).astype(ml_dtypes.bfloat16))
    blmp = np.zeros((VPAD,), np.float32)
    blmp[:V] = blm_e

    def packv(v):  # [.., N] -> [.., P, N//P]
        v = np.asarray(v, np.float32)
        nch = v.shape[-1] // P
        return np.ascontiguousarray(
            v.reshape(v.shape[:-1] + (nch, P)).swapaxes(-1, -2))

    # diagonal-block causal triangle: same on every core and every chunk
    ml_m = (np.arange(P)[:, None] <= np.arange(P)[None, :]).astype(np.float32)

    shared = dict(
        wq=pack_sq(wq_e), wk=pack_sq(wk_e), wv=pack_sq(wv_e),
        wo=pack_sq(np.asarray(gi["wo"], np.float32)),
        w1=w1p, w2=w2p, wlm=wlmp,
        bq=packv(bq_v), bk=packv(bk_v), bo=packv(bo_e),
        b1=packv(b1_e), b2=packv(np.asarray(gi["b2"], np.float32)),
        blm=np.ascontiguousarray(blmp),
        maskl=ml_m.astype(ml_dtypes.bfloat16),
    )

    in_maps = []
    for c in range(8):
        b, half = c // 2, c % 2
        sl = slice(half * TPC, (half + 1) * TPC)
        im = dict(shared)
        im["xembT"] = np.ascontiguousarray(xemb[b, sl].T, dtype=np.float32)
        im["pbias"] = np.full((P, 1), 0.0 if half else -60000.0, np.float32)
        in_maps.append(im)

    res = run_bass_kernel_spmd(nc, in_maps, list(range(8)),
                               trace=bool(os.environ.get("BASS_TRACE")))
    LAST_EXEC_NS = res.exec_time_ns

    out = np.empty((B, T, V), np.float32)
    for c in range(8):
        b, half = c // 2, c % 2
        out[b, half * TPC:(half + 1) * TPC] = res.results[c]["out"].astype(np.float32)
    return out


# revision 15
# speedup vs baseline: 1.3661x; 1.0122x over previous
"""GPT-2 (L=8, D=1024, H=16, V=50257, B=4, T=1024) forward on 8 TRN2 NeuronCores.

Sharding: core c handles batch b=c//2, sequence half h=c%2 (512 tokens).
Weights replicated (bf16). Per layer, K/V are exchanged between the two cores
of a batch-pair with an AllReduce(add); each core recovers the peer half by
subtracting its own contribution (bf16 sub). Attention chunk order is
core-relative: chunks 0-3 = local keys (direct from SBUF, no collective wait),
chunks 4-7 = peer keys. Causality is data-driven: a diagonal [128,4,512] mask
(identical on all cores) for the local half, and a per-core exp bias
(0 or -60000) that zeroes the whole peer half on first-half cores.

LN weights/biases are folded into the adjacent projection weights host-side,
so on-chip LN is a pure (x-mu)*rstd; stats are accumulated chunk-by-chunk as
the residual stream is produced. LM head runs in vocab groups of 6 sharing
the stationary activations across 6 PSUM banks, bf16 output (host upcasts).
"""

import os
import sys
import types
from contextlib import ExitStack

import numpy as np
import ml_dtypes

import concourse.bass as bass
import concourse.mybir as mybir
import concourse.tile as tile
from concourse import bacc
from concourse.bass_utils import run_bass_kernel_spmd

f32 = mybir.dt.float32
bf16 = mybir.dt.bfloat16
AF = mybir.ActivationFunctionType
OP = mybir.AluOpType

L, D, H, V, DFF = 8, 1024, 16, 50257, 4096
HS = D // H          # 64
B, T = 4, 1024
TPC = 512            # tokens per core
P = 128
DC = D // P          # 8 d-chunks
FC = DFF // P        # 32 dff-chunks
NVC = (V + 511) // 512   # 99 vocab chunks
GV = 3               # lm-head vocab chunks per group
NG = (NVC + GV - 1) // GV        # 17 groups
NVC2 = NG * GV                   # 102 (padded)
VPAD = NVC2 * 512
EPS = 1e-5
VW = H * (HS + 1)    # 1040

K_SZ = P * DC * TPC           # 524288
V_SZ = P * 4 * VW             # 532480

LAST_EXEC_NS = None
_CACHE = {}


def _install_ntff_hook():
    """Provide antenv.axon_hooks if the image lacks it, so trace=True works."""
    try:
        import antenv
        try:
            from antenv import axon_hooks  # noqa: F401
            return
        except ImportError:
            pass
        hooks_mod = types.ModuleType("antenv.axon_hooks")
        _hook = [None]
        hooks_mod.set_axon_ntff_profile_hook = lambda h: _hook.__setitem__(0, h)
        hooks_mod.get_axon_ntff_profile_hook = lambda: _hook[0]
        sys.modules["antenv.axon_hooks"] = hooks_mod
        antenv.axon_hooks = hooks_mod
        from trn_agent_boot.trn_boot import _ntff_profile_via_ctypes
        hooks_mod.set_axon_ntff_profile_hook(
            _ntff_profile_via_ctypes("/opt/axon/libaxon_pjrt.so"))
    except Exception:
        pass


def _build():
    nc = bacc.Bacc(None, target_bir_lowering=False, debug=False)

    xembT = nc.dram_tensor("xembT", [D, TPC], f32, kind="ExternalInput")
    wq = nc.dram_tensor("wq", [L, P, DC, D], bf16, kind="ExternalInput")
    wk = nc.dram_tensor("wk", [L, P, DC, D], bf16, kind="ExternalInput")
    wv = nc.dram_tensor("wv", [L, P, DC, D], bf16, kind="ExternalInput")
    wo = nc.dram_tensor("wo", [L, P, DC, D], bf16, kind="ExternalInput")
    w1 = nc.dram_tensor("w1", [L, FC, P, DC, P], bf16, kind="ExternalInput")
    w2 = nc.dram_tensor("w2", [L, DC, P, FC, P], bf16, kind="ExternalInput")
    wlm = nc.dram_tensor("wlm", [NG, P, DC, GV * 512], bf16, kind="ExternalInput")
    bq_d = nc.dram_tensor("bq", [L, P, DC], f32, kind="ExternalInput")
    bk_d = nc.dram_tensor("bk", [L, P, DC], f32, kind="ExternalInput")
    bo_d = nc.dram_tensor("bo", [L, P, DC], f32, kind="ExternalInput")
    b1_d = nc.dram_tensor("b1", [L, P, FC], f32, kind="ExternalInput")
    b2_d = nc.dram_tensor("b2", [L, P, DC], f32, kind="ExternalInput")
    blm_d = nc.dram_tensor("blm", [VPAD], f32, kind="ExternalInput")
    maskl_d = nc.dram_tensor("maskl", [P, P], bf16, kind="ExternalInput")
    pbias_d = nc.dram_tensor("pbias", [P, 1], f32, kind="ExternalInput")
    out_d = nc.dram_tensor("out", [TPC, V], bf16, kind="ExternalOutput")

    kvloc_k = nc.dram_tensor("kvloc_k", [K_SZ], bf16)
    kvred_k = nc.dram_tensor("kvred_k", [K_SZ], bf16)
    kvloc_v = nc.dram_tensor("kvloc_v", [V_SZ], bf16)
    kvred_v = nc.dram_tensor("kvred_v", [V_SZ], bf16)
    groups = [[0, 1], [2, 3], [4, 5], [6, 7]]

    with tile.TileContext(nc) as tc:
        with (
            tc.tile_pool(name="cpool", bufs=1) as cpool,
            tc.tile_pool(name="csm", bufs=2) as csm,
        ):
            # ---- persistent / common tiles
            x = cpool.tile([P, DC, TPC], f32, name="x")
            xf = cpool.tile([P, DC, TPC], bf16, name="xf")
            maskl = cpool.tile([P, P], bf16, name="maskl")
            pb = cpool.tile([P, 1], f32, name="pb")
            ones1 = cpool.tile([1, P], f32, name="ones1")
            ones128b = cpool.tile([P, 1], bf16, name="ones128b")
            eps_t = cpool.tile([1, 1], f32, name="eps_t")
            nc.vector.memset(ones1[:], 1.0)
            nc.vector.memset(ones128b[:], 1.0)
            nc.vector.memset(eps_t[:], EPS)
            nc.sync.dma_start(maskl[:], maskl_d[:])
            nc.sync.dma_start(pb[:], pbias_d[:])
            nc.sync.dma_start(x[:], xembT.rearrange("(c p) t -> p c t", p=P))

            lstack = ExitStack()
            lpool = lstack.enter_context(tc.tile_pool(name="lpool", bufs=1))
            wbig = lstack.enter_context(tc.tile_pool(name="wbig", bufs=3))
            w1p = lstack.enter_context(tc.tile_pool(name="w1p", bufs=3))
            w2p = lstack.enter_context(tc.tile_pool(name="w2p", bufs=2))
            sexpp = lstack.enter_context(tc.tile_pool(name="sexpp", bufs=4))
            sumc = lstack.enter_context(tc.tile_pool(name="sumc", bufs=2))
            vsmc = lstack.enter_context(tc.tile_pool(name="vsmc", bufs=1))
            xcp = lstack.enter_context(tc.tile_pool(name="xcp", bufs=2))
            rsmp = lstack.enter_context(tc.tile_pool(name="rsmp", bufs=1))
            small = lstack.enter_context(tc.tile_pool(name="small", bufs=2))
            small1 = lstack.enter_context(tc.tile_pool(name="small1", bufs=1))
            lnb = lstack.enter_context(tc.tile_pool(name="lnb", bufs=2))
            pscore = lstack.enter_context(tc.tile_pool(name="pscore", bufs=3, space="PSUM"))
            pav = lstack.enter_context(tc.tile_pool(name="pav", bufs=2, space="PSUM"))
            pmm = lstack.enter_context(tc.tile_pool(name="pmm", bufs=2, space="PSUM"))
            pstat = lstack.enter_context(tc.tile_pool(name="pstat", bufs=1, space="PSUM"))
            if True:
                h = lpool.tile([P, DC, TPC], bf16, name="h")
                qbf = lpool.tile([P, DC, TPC], bf16, name="qbf")
                kst = lpool.tile([P, DC, TPC], bf16, name="kst")
                kpeer = lpool.tile([P, DC, TPC], bf16, name="kpeer")
                vst = lpool.tile([P, 4, VW], bf16, name="vst")
                vpeer = lpool.tile([P, 4, VW], bf16, name="vpeer")
                obf = lpool.tile([P, DC, TPC], bf16, name="obf")
                xbf = lpool.tile([P, DC, TPC], bf16, name="xbf")
                r = lpool.tile([P, 16, TPC], bf16, name="r")
                statp = pstat.tile([P, TPC], f32, tag="st", name="statp")
                sxp = statp[0:1, :]
                sqp = statp[64:65, :]
                # ones columns of V_aug, set once (data writes never touch them)
                nc.vector.memset(vst[:], 1.0)

                def stats_chunk(m, first, last, eng):
                    eng.tensor_copy(xbf[:, m, :], x[:, m, :])
                    sqb = xcp.tile([P, TPC], bf16, tag="sq", name=f"sqb_{m}")
                    nc.vector.tensor_mul(sqb[:], xbf[:, m, :], xbf[:, m, :])
                    nc.tensor.matmul(sxp, ones128b[:], xbf[:, m, :], start=first, stop=last,
                                     skip_group_check=True)
                    nc.tensor.matmul(sqp, ones128b[:], sqb[:], start=first, stop=last,
                                     skip_group_check=True)

                def ln_finish(nm):
                    mu = small1.tile([1, TPC], f32, tag="mu", name=f"mu_{nm}")
                    ex2 = small1.tile([1, TPC], f32, tag="ex2", name=f"ex2_{nm}")
                    nc.vector.tensor_scalar_mul(mu[:], sxp, 1.0 / D)
                    nc.vector.tensor_scalar_mul(ex2[:], sqp, 1.0 / D)
                    var = small1.tile([1, TPC], f32, tag="var", name=f"var_{nm}")
                    nc.vector.tensor_mul(var[:], mu[:], mu[:])
                    nc.vector.tensor_sub(var[:], ex2[:], var[:])
                    nc.scalar.activation(var[:], var[:], AF.Sqrt, bias=eps_t[:], scale=1.0)
                    rstd = small1.tile([1, TPC], f32, tag="rstd", name=f"rstd_{nm}")
                    nc.vector.reciprocal(rstd[:], var[:])
                    msb2 = small1.tile([1, TPC], f32, tag="msb2", name=f"msb2_{nm}")
                    nc.vector.tensor_mul(msb2[:], mu[:], rstd[:])
                    bc1 = pmm.tile([P, TPC], f32, tag="mm", name=f"bc1_{nm}")
                    nc.tensor.matmul(bc1[:], ones1[:], rstd[:], start=True, stop=True)
                    rsb = rsmp.tile([P, TPC], bf16, tag="rsb", name=f"rsb_{nm}")
                    nc.scalar.copy(rsb[:], bc1[:])
                    bc2 = pmm.tile([P, TPC], f32, tag="mm", name=f"bc2_{nm}")
                    nc.tensor.matmul(bc2[:], ones1[:], msb2[:], start=True, stop=True)
                    msb = rsmp.tile([P, TPC], bf16, tag="msb", name=f"msb_{nm}")
                    nc.scalar.copy(msb[:], bc2[:])
                    return rsb, msb

                def ln_apply(out_bf, rsb, msb):
                    for hf in range(2):
                        sl = slice(hf * 4, hf * 4 + 4)
                        nc.vector.tensor_mul(out_bf[:, sl, :], xbf[:, sl, :],
                                             rsb[:, None, :].to_broadcast([P, 4, TPC]))
                        nc.vector.tensor_sub(out_bf[:, sl, :], out_bf[:, sl, :],
                                             msb[:, None, :].to_broadcast([P, 4, TPC]))

                # ---- initial LN1 (layer 0)
                for m in range(DC):
                    stats_chunk(m, m == 0, m == DC - 1,
                                nc.gpsimd if m % 2 else nc.vector)
                rsb0, msb0 = ln_finish("l0")
                ln_apply(h, rsb0, msb0)

                for li in range(L):
                    bqt = lnb.tile([P, DC, 1], f32, tag="bq", name=f"bqt_{li}")
                    nc.sync.dma_start(bqt[:], bq_d[li][:, :, None])
                    bkt = lnb.tile([P, DC, 1], f32, tag="bk", name=f"bkt_{li}")
                    nc.sync.dma_start(bkt[:], bk_d[li][:, :, None])

                    # ---------- weight prefetch (scalar DGE queue) ----------
                    wkhs, wqhs, wvhs = [], [], []
                    for hf in range(2):
                        wkh = wbig.tile([P, DC, 512], bf16, tag="w", name=f"wk_{li}_{hf}")
                        nc.scalar.dma_start(wkh[:], wk[li, :, :, hf * 512:(hf + 1) * 512])
                        wkhs.append(wkh)
                    for hf in range(2):
                        wqh = wbig.tile([P, DC, 512], bf16, tag="w", name=f"wq_{li}_{hf}")
                        nc.scalar.dma_start(wqh[:], wq[li, :, :, hf * 512:(hf + 1) * 512])
                        wqhs.append(wqh)
                    for hf in range(2):
                        wvh = wbig.tile([P, DC, 512], bf16, tag="w", name=f"wv_{li}_{hf}")
                        nc.scalar.dma_start(wvh[:], wv[li, :, :, hf * 512:(hf + 1) * 512])
                        wvhs.append(wvh)

                    # ---------- K projection, stage, collective ----------
                    for hf in range(2):
                        for mm_ in range(4):
                            m = hf * 4 + mm_
                            ps = pmm.tile([P, TPC], f32, tag="mm", name=f"kps_{li}_{m}")
                            for c in range(DC):
                                nc.tensor.matmul(ps[:], wkhs[hf][:, c, mm_ * P:(mm_ + 1) * P],
                                                 h[:, c, :], start=(c == 0), stop=(c == DC - 1))
                            nc.scalar.activation(kst[:, m, :], ps[:], AF.Identity, bias=bkt[:, m])
                    nc.sync.dma_start(
                        kvloc_k.rearrange("(p c t) -> p c t", c=DC, t=TPC), kst[:])
                    nc.gpsimd.collective_compute(
                        "AllReduce", OP.add, replica_groups=groups,
                        ins=[kvloc_k[:]], outs=[kvred_k[:]])

                    # ---------- Q projection ----------
                    for hf in range(2):
                        for mm_ in range(4):
                            m = hf * 4 + mm_
                            ps = pmm.tile([P, TPC], f32, tag="mm", name=f"qps_{li}_{m}")
                            for c in range(DC):
                                nc.tensor.matmul(ps[:], wqhs[hf][:, c, mm_ * P:(mm_ + 1) * P],
                                                 h[:, c, :], start=(c == 0), stop=(c == DC - 1))
                            nc.scalar.activation(qbf[:, m, :], ps[:], AF.Identity, bias=bqt[:, m])

                    # ---------- V projection, stage, collective ----------
                    for hf in range(2):
                        for tc4 in range(4):
                            ps = pmm.tile([P, TPC], f32, tag="mm", name=f"vps_{li}_{hf}_{tc4}")
                            for c in range(DC):
                                nc.tensor.matmul(
                                    ps[:], h[:, c, tc4 * P:(tc4 + 1) * P],
                                    wvhs[hf][:, c, :], start=(c == 0), stop=(c == DC - 1))
                            dst = vst[:, tc4, :].rearrange("p (h e) -> p h e", e=HS + 1)
                            if tc4 % 2:
                                nc.vector.tensor_copy(
                                    dst[:, hf * 8:(hf + 1) * 8, 0:HS],
                                    ps[:].rearrange("p (h e) -> p h e", e=HS))
                            else:
                                nc.scalar.copy(
                                    dst[:, hf * 8:(hf + 1) * 8, 0:HS],
                                    ps[:].rearrange("p (h e) -> p h e", e=HS))
                    nc.sync.dma_start(
                        kvloc_v.rearrange("(p c t) -> p c t", c=4, t=VW), vst[:])
                    nc.gpsimd.collective_compute(
                        "AllReduce", OP.add, replica_groups=groups,
                        ins=[kvloc_v[:]], outs=[kvred_v[:]])

                    # ---------- peer K/V recovery (chunked) ----------
                    for c in range(DC):
                        ks = sumc.tile([P, TPC], bf16, tag="ks", name=f"ks_{li}_{c}")
                        nc.sync.dma_start(
                            ks[:], kvred_k.rearrange("(p c t) -> p c t", c=DC, t=TPC)[:, c, :])
                        nc.vector.tensor_sub(kpeer[:, c, :], ks[:], kst[:, c, :])
                    for tc4 in range(4):
                        vs = vsmc.tile([P, VW], bf16, tag="vs", name=f"vs_{li}_{tc4}")
                        nc.sync.dma_start(
                            vs[:], kvred_v.rearrange("(p c t) -> p c t", c=4, t=VW)[:, tc4, :])
                        nc.vector.tensor_sub(vpeer[:, tc4, :], vs[:], vst[:, tc4, :])

                    # ---------- attention ----------
                    def head_scores(hd):
                        hp = (hd % 2) * HS
                        hc = hd // 2
                        sexp = sexpp.tile([P, DC, TPC], bf16, tag="sx", name=f"sexp_{li}_{hd}")
                        for kt in range(4):
                            nq = TPC - kt * P
                            ps = pscore.tile([P, TPC], f32, tag="sc", name=f"sL_{li}_{hd}_{kt}")
                            nc.tensor.matmul(ps[:, 0:nq], kst[hp:hp + HS, hc, kt * P:(kt + 1) * P],
                                             qbf[hp:hp + HS, hc, kt * P:], start=True, stop=True)
                            nc.scalar.activation(sexp[:, kt, kt * P:], ps[:, 0:nq],
                                                 AF.Exp, scale=HS ** -0.5)
                            nc.vector.tensor_mul(sexp[:, kt, kt * P:(kt + 1) * P],
                                                 sexp[:, kt, kt * P:(kt + 1) * P], maskl[:])
                        for kt in range(4):
                            ps = pscore.tile([P, TPC], f32, tag="sc", name=f"sR_{li}_{hd}_{kt}")
                            nc.tensor.matmul(ps[:], kpeer[hp:hp + HS, hc, kt * P:(kt + 1) * P],
                                             qbf[hp:hp + HS, hc, :], start=True, stop=True)
                            nc.scalar.activation(sexp[:, 4 + kt, :], ps[:], AF.Exp,
                                                 scale=HS ** -0.5, bias=pb[:])
                        return sexp

                    def head_av(hd, sexp):
                        hp = (hd % 2) * HS
                        hc = hd // 2
                        av = pav.tile([P, TPC], f32, tag="av", name=f"av_{li}_{hd}")
                        for kt in range(4):
                            nc.tensor.matmul(av[0:HS + 1, kt * P:], vst[:, kt, hd * 65:hd * 65 + 65],
                                             sexp[:, kt, kt * P:], start=(kt == 0), stop=False,
                                             skip_group_check=True)
                        for kt in range(4):
                            nc.tensor.matmul(av[0:HS + 1, :], vpeer[:, kt, hd * 65:hd * 65 + 65],
                                             sexp[:, 4 + kt, :], start=False, stop=(kt == 3),
                                             skip_group_check=True)
                        rc = small.tile([1, TPC], f32, tag="rc", name=f"rc_{li}_{hd}")
                        nc.vector.reciprocal(rc[:], av[HS:HS + 1, :])
                        return av, rc

                    def head_norm(hd, av, rc):
                        hp = (hd % 2) * HS
                        hc = hd // 2
                        bc = pmm.tile([P, TPC], f32, tag="mm", name=f"bcp_{li}_{hd}")
                        nc.tensor.matmul(bc[0:HS, :], ones1[:, 0:HS], rc[:], start=True, stop=True)
                        bcs = small.tile([HS, TPC], f32, tag="bcs", name=f"bcs_{li}_{hd}")
                        nc.vector.tensor_copy(bcs[:], bc[0:HS, :])
                        nc.vector.tensor_mul(obf[hp:hp + HS, hc, :], av[0:HS, :], bcs[:])

                    se_p = av_p = None
                    for hd in range(H + 2):
                        se = head_scores(hd) if hd < H else None
                        if se_p is not None:
                            av_n = (hd - 1,) + head_av(hd - 1, se_p)
                        else:
                            av_n = None
                        if av_p is not None:
                            head_norm(*av_p)
                        se_p, av_p = se, av_n

                    # ---------- O projection + residual + LN2 stats ----------
                    bot = lnb.tile([P, DC, 1], f32, tag="bo", name=f"bot_{li}")
                    nc.sync.dma_start(bot[:], bo_d[li][:, :, None])
                    for hf in range(2):
                        woh = wbig.tile([P, DC, 512], bf16, tag="w", name=f"wo_{li}_{hf}")
                        nc.scalar.dma_start(woh[:], wo[li, :, :, hf * 512:(hf + 1) * 512])
                        for mm_ in range(4):
                            m = hf * 4 + mm_
                            ps = pmm.tile([P, TPC], f32, tag="mm", name=f"ops_{li}_{m}")
                            for c in range(DC):
                                nc.tensor.matmul(ps[:], woh[:, c, mm_ * P:(mm_ + 1) * P],
                                                 obf[:, c, :], start=(c == 0), stop=(c == DC - 1))
                            nc.vector.scalar_tensor_tensor(
                                x[:, m, :], ps[:], bot[:, m], x[:, m, :], op0=OP.add, op1=OP.add)
                            stats_chunk(m, m == 0, m == DC - 1,
                                        nc.gpsimd if m % 2 else nc.vector)
                    rsb2, msb2s = ln_finish(f"l2_{li}")
                    ln_apply(h, rsb2, msb2s)

                    # ---------- MLP (two halves of DFF) ----------
                    b1t = lnb.tile([P, FC, 1], f32, tag="b1", name=f"b1t_{li}")
                    nc.sync.dma_start(b1t[:], b1_d[li][:, :, None])
                    b2t = lnb.tile([P, DC, 1], f32, tag="b2", name=f"b2t_{li}")
                    nc.sync.dma_start(b2t[:], b2_d[li][:, :, None])
                    for fh in range(2):
                        for mfl in range(16):
                            mf = fh * 16 + mfl
                            w1t = w1p.tile([P, DC, P], bf16, tag="w1", name=f"w1_{li}_{mf}")
                            nc.scalar.dma_start(w1t[:], w1[li, mf])
                            ps = pmm.tile([P, TPC], f32, tag="mm", name=f"mps_{li}_{mf}")
                            for c in range(DC):
                                nc.tensor.matmul(ps[:], w1t[:, c, :], h[:, c, :],
                                                 start=(c == 0), stop=(c == DC - 1))
                            nc.scalar.activation(r[:, mfl, :], ps[:], AF.Relu,
                                                 bias=b1t[:, mf], scale=1.0)
                        for m in range(DC):
                            w2t = w2p.tile([P, 16, P], bf16, tag="w2", name=f"w2_{li}_{fh}_{m}")
                            nc.scalar.dma_start(w2t[:], w2[li, m, :, fh * 16:(fh + 1) * 16, :])
                            ps = pmm.tile([P, TPC], f32, tag="mm", name=f"m2_{li}_{fh}_{m}")
                            for f in range(16):
                                nc.tensor.matmul(ps[:], w2t[:, f, :], r[:, f, :],
                                                 start=(f == 0), stop=(f == 15))
                            if fh == 0:
                                nc.vector.scalar_tensor_tensor(
                                    x[:, m, :], ps[:], b2t[:, m], x[:, m, :],
                                    op0=OP.add, op1=OP.add)
                            else:
                                nc.vector.tensor_add(x[:, m, :], x[:, m, :], ps[:])
                                stats_chunk(m, m == 0, m == DC - 1,
                                            nc.gpsimd if m % 2 else nc.vector)
                    if li < L - 1:
                        rsb1, msb1 = ln_finish(f"l1_{li + 1}")
                        ln_apply(h, rsb1, msb1)

                # ---------- final LN ----------
                rsbf, msbf = ln_finish("lf")
                ln_apply(xf, rsbf, msbf)

            # ---------- LM head ----------
            lstack.close()
            lmstack = ExitStack()
            wg = lmstack.enter_context(tc.tile_pool(name="wg", bufs=3))
            otp = lmstack.enter_context(tc.tile_pool(name="otp", bufs=4))
            blsp = lmstack.enter_context(tc.tile_pool(name="blsp", bufs=2))
            pacc = lmstack.enter_context(tc.tile_pool(name="pacc", bufs=4, space="PSUM"))
            pbc = lmstack.enter_context(tc.tile_pool(name="pbc", bufs=2, space="PSUM"))
            if True:
                for g in range(NG):
                    g0 = g * GV
                    gn = GV
                    wgt = wg.tile([P, DC, GV * 512], bf16, tag="wg", name=f"wg_{g}")
                    nc.scalar.dma_start(wgt[:], wlm[g])
                    blg = blsp.tile([1, GV * 512], f32, tag="blg", name=f"blg_{g}")
                    nc.sync.dma_start(blg[:], blm_d[None, g0 * 512:(g0 + gn) * 512])
                    blsts = []
                    for vi in range(gn):
                        bcp = pbc.tile([P, 512], f32, tag="bc", name=f"bcp_{g}_{vi}")
                        nc.tensor.matmul(bcp[:], ones1[:], blg[0:1, vi * 512:(vi + 1) * 512],
                                         start=True, stop=True)
                        blst = blsp.tile([P, 512], f32, tag=f"bls{vi}", name=f"bls_{g}_{vi}")
                        nc.scalar.activation(blst[:], bcp[:], AF.Copy)
                        blsts.append(blst)
                    for tc4 in range(4):
                        for vi in range(gn):
                            vc = g0 + vi
                            ps = pacc.tile([P, 512], f32, tag="acc", name=f"lm_{g}_{tc4}_{vi}")
                            for c in range(DC):
                                nc.tensor.matmul(
                                    ps[:], xf[:, c, tc4 * P:(tc4 + 1) * P],
                                    wgt[:, c, vi * 512:(vi + 1) * 512],
                                    start=(c == 0), stop=(c == DC - 1))
                            if vc >= NVC:
                                continue
                            nv = min(512, V - vc * 512)
                            ott = otp.tile([P, 512], bf16, tag="ot", name=f"ot_{g}_{tc4}_{vi}")
                            nc.vector.scalar_tensor_tensor(
                                ott[:], ps[:], 1.0, blsts[vi][:],
                                op0=OP.mult, op1=OP.add)
                            nc.sync.dma_start(
                                out_d[tc4 * P:(tc4 + 1) * P, vc * 512:vc * 512 + nv],
                                ott[:, 0:nv])
            lmstack.close()

    nc.compile()
    return nc


def kernel(**inputs):
    global LAST_EXEC_NS
    _install_ntff_hook()
    if "nc" not in _CACHE:
        _CACHE["nc"] = _build()
    nc = _CACHE["nc"]

    gi = {k: np.asarray(v, np.float32) if np.asarray(v).dtype == np.float32
          else np.asarray(v) for k, v in inputs.items()}
    idx = np.asarray(gi["idx"]).astype(np.int64)
    xemb = np.asarray(gi["wte"])[idx] + np.asarray(gi["wpe"])[:T][None, :, :]

    # ---- fold LN weights/biases into adjacent projections (host, fp32)
    ln1w = np.asarray(gi["ln1_w"]); ln1b = np.asarray(gi["ln1_b"])
    ln2w = np.asarray(gi["ln2_w"]); ln2b = np.asarray(gi["ln2_b"])
    lnfw = np.asarray(gi["lnf_w"]); lnfb = np.asarray(gi["lnf_b"])
    wq_e = ln1w[:, :, None] * gi["wq"]          # [L,D,D]
    wk_e = ln1w[:, :, None] * gi["wk"]
    wv_e = ln1w[:, :, None] * gi["wv"]
    bq_v = np.einsum('ld,lde->le', ln1b, gi["wq"])   # [L,D]
    bk_v = np.einsum('ld,lde->le', ln1b, gi["wk"])
    bv_v = np.einsum('ld,lde->le', ln1b, gi["wv"])
    bo_e = gi["bo"] + np.einsum('ld,lde->le', bv_v, gi["wo"])
    w1_e = ln2w[:, :, None] * gi["w1"]
    b1_e = gi["b1"] + np.einsum('ld,lde->le', ln2b, gi["w1"])
    wlm_e = lnfw[:, None] * gi["wlm"]
    blm_e = gi["blm"] + lnfb @ gi["wlm"]

    def pack_sq(w):   # [L, 1024, N] -> [L, 128, 8, N]
        Lw, Kw, Nw = w.shape
        return np.ascontiguousarray(
            w.reshape(Lw, DC, P, Nw).transpose(0, 2, 1, 3).astype(ml_dtypes.bfloat16))

    w1p = np.ascontiguousarray(
        w1_e.reshape(L, DC, P, FC, P).transpose(0, 3, 2, 1, 4).astype(ml_dtypes.bfloat16))
    w2p = np.ascontiguousarray(
        np.asarray(gi["w2"]).reshape(L, FC, P, DC, P).transpose(0, 3, 2, 1, 4)
        .astype(ml_dtypes.bfloat16))
    wlmp = np.zeros((D, VPAD), np.float32)
    wlmp[:, :V] = wlm_e
    wlmp = np.ascontiguousarray(
        wlmp.reshape(DC, P, NG, GV * 512).transpose(2, 1, 0, 3).astype(ml_dtypes.bfloat16))
    blmp = np.zeros((VPAD,), np.float32)
    blmp[:V] = blm_e

    def packv(v):  # [.., N] -> [.., P, N//P]
        v = np.asarray(v, np.float32)
        nch = v.shape[-1] // P
        return np.ascontiguousarray(
            v.reshape(v.shape[:-1] + (nch, P)).swapaxes(-1, -2))

    # diagonal-block causal triangle: same on every core and every chunk
    ml_m = (np.arange(P)[:, None] <= np.arange(P)[None, :]).astype(np.float32)

    shared = dict(
        wq=pack_sq(wq_e), wk=pack_sq(wk_e), wv=pack_sq(wv_e),
        wo=pack_sq(np.asarray(gi["wo"], np.float32)),
        w1=w1p, w2=w2p, wlm=wlmp,
        bq=packv(bq_v), bk=packv(bk_v), bo=packv(bo_e),
        b1=packv(b1_e), b2=packv(np.asarray(gi["b2"], np.float32)),
        blm=np.ascontiguousarray(blmp),
        maskl=ml_m.astype(ml_dtypes.bfloat16),
    )

    in_maps = []
    for c in range(8):
        b, half = c // 2, c % 2
        sl = slice(half * TPC, (half + 1) * TPC)
        im = dict(shared)
        im["xembT"] = np.ascontiguousarray(xemb[b, sl].T, dtype=np.float32)
        im["pbias"] = np.full((P, 1), 0.0 if half else -60000.0, np.float32)
        in_maps.append(im)

    res = run_bass_kernel_spmd(nc, in_maps, list(range(8)),
                               trace=bool(os.environ.get("BASS_TRACE")))
    LAST_EXEC_NS = res.exec_time_ns

    out = np.empty((B, T, V), np.float32)
    for c in range(8):
        b, half = c // 2, c % 2
        out[b, half * TPC:(half + 1) * TPC] = res.results[c]["out"].astype(np.float32)
    return out
